# revision 6
# baseline (speedup 1.0000x reference)
"""Causal multi-head attention (B=2, S=2048, D=1024, H=16) on 8 trn2 cores.

Sharding: core c handles heads {4i..4i+3} (i = c%4) of batch c//4 only.
All matmul operands are bfloat16 (PSUM accumulates fp32). The output
projection is row-parallel: each core multiplies its own heads' attention
output by the matching 256 rows of Wout, producing a bf16 partial
[2048, 1024] for its batch; the host unshards by summing the four
partials per batch and adding bout. No device collective is needed.

Per core:
  - project host-pretransposed x_b^T [D, S] through the core's Wqkv
    column slice into Q^T/K^T head-pair tiles and V (natural layout with
    a fused ones-column so the AV matmul emits softmax denominators),
  - causal attention per head pair in transposed layout: scores^T = K Q^T
    (row-tiled head pair), exp on ScalarE, causal diagonal-band masks via
    gpsimd affine_select (band kept 128 wide so the predicate iota stays
    exact in bf16), A^T V on PE, normalization straight out of PSUM.

Schedule: the ScalarE exp stream (~73us) is nearly as long as the whole
attention-phase PE work, so the PE stream is issued as
scores(kc) / AV(kc-LAG) interleaved -- the PE then tracks the exp stream
instead of racing ahead into PSUM-bank stalls. The previous q-block's
output projection is threaded into the next block's scores window, and
out tiles are stored as full [128, 1024] rows with one DMA each.
"""

import sys

for _p in ("/opt/trn_rl_repo", "/opt/pypackages"):
    if _p not in sys.path:
        sys.path.insert(0, _p)

import numpy as np
import ml_dtypes

import concourse.bass as bass
import concourse.mybir as mybir
import concourse.tile as tile
from concourse import bacc
from concourse.bass_utils import run_bass_kernel_spmd

B = 2
S = 2048
D = 1024
H = 16
DH = 64
NCORES = 8
SB = 512           # q block (matmul moving dim)
KC = 128           # k chunk (contraction tile)
NSB = S // SB      # 4 q-blocks
NKC = S // KC      # 16 k-chunks
NDC = D // KC      # 8 contraction chunks for the projections
NCS = SB // KC     # 4 k-chunks per sequence block
NP = 2             # head pairs per core
LAG = 3            # AV trails scores by LAG chunks in the PE stream

_compiled = None


def _build():
    f32 = mybir.dt.float32
    bf16 = mybir.dt.bfloat16
    nc = bacc.Bacc(None, target_bir_lowering=False)

    # host-blocked inputs: every [128, N] tile is contiguous in DRAM.
    xt = nc.declare_dram_parameter("xt", [NSB, NDC, KC, SB], bf16, isOutput=False)
    wqk = nc.declare_dram_parameter("wqk", [NP, NDC, KC, 2 * KC], bf16,
                                    isOutput=False)
    wv = nc.declare_dram_parameter("wv", [NDC, KC, 2 * KC], bf16, isOutput=False)
    wout = nc.declare_dram_parameter("wout", [NP, KC, D], bf16, isOutput=False)
    bqk = nc.declare_dram_parameter("bqk", [NP, 2, KC], f32, isOutput=False)
    bv = nc.declare_dram_parameter("bv", [2 * KC], f32, isOutput=False)
    # blocked [16, 128, 1024]: one contiguous 256KB block per token chunk
    out_ext = nc.declare_dram_parameter("out", [NSB * NCS, KC, D], bf16,
                                        isOutput=True)

    with tile.TileContext(nc) as tc:
        with (
            tc.tile_pool(name="qkv", bufs=1) as qkvp,
            tc.tile_pool(name="obuf", bufs=1) as op,
            tc.tile_pool(name="misc", bufs=1) as mp,
            tc.tile_pool(name="evict", bufs=1) as ep,
        ):
            # ---- small constants -----------------------------------------
            bqk_t = [[mp.tile([KC, 1], f32, tag=f"bqk{hp}_{m}",
                              name=f"bqk{hp}_{m}") for m in range(2)]
                     for hp in range(NP)]
            for hp in range(NP):
                for m in range(2):
                    nc.scalar.dma_start(
                        out=bqk_t[hp][m][:],
                        in_=bqk[hp, m].rearrange("(p o) -> p o", o=1),
                    )
            bv_row = mp.tile([1, 2 * KC], f32, tag="bv_row")
            nc.scalar.dma_start(out=bv_row[:], in_=bv.rearrange("(o f) -> o f", o=1))
            bv_bc = mp.tile([KC, 2 * KC], f32, tag="bv_bc")
            nc.gpsimd.partition_broadcast(out_ap=bv_bc[:], in_ap=bv_row[:])

            # warm the ScalarE Exp table before the first real exp
            warm = mp.tile([1, 4], f32, tag="warm")
            nc.vector.memset(warm[:], 0.0)
            nc.scalar.activation(warm[:], warm[:],
                                 mybir.ActivationFunctionType.Exp)

            # Wout row slices for my two head pairs
            wout_t = [mp.tile([KC, D], bf16, tag=f"wo{hp}", name=f"wo{hp}")
                      for hp in range(NP)]

            # ---- persistent activations ----------------------------------
            # pair hp = heads (4i+2hp, 4i+2hp+1) of my batch.
            # QQ[hp]: rows 0:64 = Q^T of the even head, rows 64:128 odd head
            QQ = [[qkvp.tile([KC, SB], bf16, tag=f"QQ{hp}_{s}", name=f"QQ{hp}_{s}")
                   for s in range(NSB)] for hp in range(NP)]
            KK = [[qkvp.tile([KC, SB], bf16, tag=f"KK{hp}_{s}", name=f"KK{hp}_{s}")
                   for s in range(NSB)] for hp in range(NP)]
            # V[hp][s]: [128, 4*2*65]; chunk sc head hh at cols
            # (sc*2+hh)*65 .. +64; col +64 holds 1.0 (denominator trick)
            V = [[qkvp.tile([KC, NCS * 2 * (DH + 1)], bf16, tag=f"V{hp}_{s}",
                            name=f"V{hp}_{s}")
                  for s in range(NSB)] for hp in range(NP)]
            for hp in range(NP):
                for s in range(NSB):
                    vv = V[hp][s][:].rearrange("p (k h c) -> p k h c", h=2,
                                               c=DH + 1)
                    nc.vector.memset(vv[:, :, :, DH], 1.0)
            # O[hp]: rows 0:64 = even head out^T (normalized), 64:128 odd
            O = [op.tile([KC, S], bf16, tag=f"O{hp}", name=f"O{hp}")
                 for hp in range(NP)]

            # ---- phase 1: projections ------------------------------------
            with (
                tc.tile_pool(name="pjw", bufs=1) as wp,
                tc.tile_pool(name="xbuf", bufs=1) as xp,
                tc.tile_pool(name="psum_proj", bufs=1, space="PSUM") as pp,
            ):
                wqk_t = [[wp.tile([KC, 2 * KC], bf16, tag=f"wqk{hp}_{k}",
                                  name=f"wqk{hp}_{k}") for k in range(NDC)]
                         for hp in range(NP)]
                wv_t = [wp.tile([KC, 2 * KC], bf16, tag=f"wv{k}", name=f"wv{k}")
                        for k in range(NDC)]

                # startup-critical DMA order: x of sblk0 first, then wqk,
                # then wv (NOT deferred -- a deferred wv stalled the PE 8us),
                # then x sblk1, wout, x sblk2-3. Four issue queues.
                dma_engs = [nc.sync, nc.gpsimd, nc.scalar]
                _dma_i = [0]

                def dma(out_t, in_t):
                    dma_engs[_dma_i[0] % 3].dma_start(out=out_t, in_=in_t)
                    _dma_i[0] += 1

                xtiles = [[None] * NDC for _ in range(NSB)]
                for sblk in range(NSB):
                    for k in range(NDC):
                        xtiles[sblk][k] = xp.tile(
                            [KC, SB], bf16, tag=f"x{sblk}_{k}",
                            name=f"x{sblk}_{k}")
                for k in range(NDC):
                    dma(xtiles[0][k][:], xt[0, k])
                for hp in range(NP):
                    for k in range(NDC):
                        dma(wqk_t[hp][k][:], wqk[hp, k])
                for k in range(NDC):
                    dma(wv_t[k][:], wv[k])
                for k in range(NDC):
                    dma(xtiles[1][k][:], xt[1, k])
                for hp in range(NP):
                    dma(wout_t[hp][:], wout[hp])
                for sblk in range(2, NSB):
                    for k in range(NDC):
                        dma(xtiles[sblk][k][:], xt[sblk, k])

                for sblk in range(NSB):
                    xall = xtiles[sblk]
                    for hp in range(NP):
                        # m-chunk 0 -> QQ[hp], 1 -> KK[hp]
                        for m in range(2):
                            ps = pp.tile([KC, SB], f32, tag="ps_qk", bufs=4)
                            for k in range(NDC):
                                nc.tensor.matmul(
                                    ps[:],
                                    wqk_t[hp][k][:, m * KC:(m + 1) * KC],
                                    xall[k][:],
                                    start=(k == 0),
                                    stop=(k == NDC - 1),
                                )
                            dest = (QQ if m == 0 else KK)[hp][sblk]
                            nc.vector.tensor_scalar_add(
                                dest[:], ps[:], bqk_t[hp][m][:],
                            )
                    # V natural: lhsT = x^T chunk; rhs = Wv [128, 256]
                    for sc in range(NCS):
                        ps = pp.tile([KC, 2 * KC], f32, tag="ps_v", bufs=4)
                        for k in range(NDC):
                            nc.tensor.matmul(
                                ps[:],
                                xall[k][:, sc * KC:(sc + 1) * KC],
                                wv_t[k][:],
                                start=(k == 0),
                                stop=(k == NDC - 1),
                            )
                        for hp in range(NP):
                            vslc = V[hp][sblk][:, sc * 2 * (DH + 1):
                                               (sc + 1) * 2 * (DH + 1)]
                            vv = vslc.rearrange("p (h c) -> p h c", c=DH + 1)
                            ps2 = ps[:, hp * KC:(hp + 1) * KC].rearrange(
                                "p (h c) -> p h c", c=DH)
                            bv2 = bv_bc[:, hp * KC:(hp + 1) * KC].rearrange(
                                "p (h c) -> p h c", c=DH)
                            nc.vector.tensor_add(vv[:, :, 0:DH], ps2[:], bv2[:])

            # ---- phase 2: attention + interleaved partial out-proj -------
            with (
                tc.tile_pool(name="pbuf", bufs=1) as pb,
                tc.tile_pool(name="psum_att", bufs=1, space="PSUM") as pa,
            ):
                def emit_scores(P, hp, qblk, kc):
                    d = kc - 4 * qblk
                    # causal: columns < 128*d are fully masked; skip them
                    c0 = KC * max(d, 0)
                    ps = pa.tile([KC, 2 * SB], f32, tag="ps_s", bufs=2)
                    for hh in range(2):  # row-tiled head pair
                        r0 = hh * DH
                        nc.tensor.matmul(
                            ps[:, hh * SB + c0:(hh + 1) * SB],
                            KK[hp][kc // 4][r0:r0 + DH,
                                            (kc % 4) * KC:(kc % 4 + 1) * KC],
                            QQ[hp][qblk][r0:r0 + DH, c0:SB],
                            start=True,
                            stop=True,
                        )
                    ps3 = ps[:].rearrange("p (h f) -> p h f", h=2)
                    pd3 = P[kc][:].rearrange("p (h f) -> p h f", h=2)
                    nc.scalar.activation(
                        pd3[:, :, c0:SB],
                        ps3[:, :, c0:SB],
                        mybir.ActivationFunctionType.Exp,
                        scale=1.0 / float(np.sqrt(DH)),
                    )
                    if d >= 0:  # diagonal chunk: zero where k > q
                        # only the 128-col diagonal band needs the mask
                        nc.gpsimd.affine_select(
                            out=pd3[:, :, c0:c0 + KC],
                            in_=pd3[:, :, c0:c0 + KC],
                            pattern=[[0, 2], [1, KC]],
                            compare_op=mybir.AluOpType.is_ge,
                            fill=0.0,
                            base=0,
                            channel_multiplier=-1,
                        )

                def emit_av(P, pos, hp, qblk, kc, nkc):
                    d = kc - 4 * qblk
                    c0 = KC * max(d, 0)
                    for hh in range(2):
                        nc.tensor.matmul(
                            pos[hh][:, c0:SB],
                            V[hp][kc // 4][:,
                                ((kc % 4) * 2 + hh) * (DH + 1):
                                ((kc % 4) * 2 + hh + 1) * (DH + 1)],
                            P[kc][:, hh * SB + c0:(hh + 1) * SB],
                            start=(kc == 0),
                            stop=(kc == nkc - 1),
                        )

                def emit_norm(pos, hp, qblk):
                    for hh in range(2):
                        # custom-DVE ops can't read PSUM: evict the
                        # denominator row to SBUF first
                        den0 = ep.tile([1, SB], f32, tag="den0", bufs=2)
                        nc.vector.tensor_copy(den0[:], pos[hh][DH:DH + 1, :])
                        rden = ep.tile([1, SB], f32, tag="rden", bufs=2)
                        nc.vector.reciprocal(rden[:], den0[:])
                        rden_bc = ep.tile([DH, SB], f32, tag="rden_bc",
                                          bufs=2)
                        nc.gpsimd.partition_broadcast(
                            out_ap=rden_bc[:], in_ap=rden[:]
                        )
                        r0 = hh * DH
                        nc.vector.tensor_mul(
                            O[hp][r0:r0 + DH, qblk * SB:(qblk + 1) * SB],
                            pos[hh][0:DH, :],
                            rden_bc[:],
                        )

                # out-projection work units for one q-block; emitted lazily
                # so they thread into the NEXT q-block's scores window
                def make_outproj_units(qblk):
                    units = []
                    otiles = {}

                    def unit(tc_, nb):
                        def emit():
                            t0 = qblk * SB + tc_ * KC
                            if nb == 0:
                                otiles[tc_] = ep.tile(
                                    [KC, D], bf16, tag="osb", bufs=2,
                                    name=f"osb{qblk}_{tc_}")
                            pso = pa.tile([KC, SB], f32, tag="ps_o", bufs=2,
                                          name=f"pso{qblk}_{tc_}_{nb}")
                            for hp in range(NP):
                                nc.tensor.matmul(
                                    pso[:],
                                    O[hp][:, t0:t0 + KC],
                                    wout_t[hp][:, nb * SB:(nb + 1) * SB],
                                    start=(hp == 0),
                                    stop=(hp == NP - 1),
                                )
                            nc.vector.tensor_copy(
                                otiles[tc_][:, nb * SB:(nb + 1) * SB], pso[:])
                            if nb == 1:
                                nc.sync.dma_start(
                                    out=out_ext[qblk * NCS + tc_],
                                    in_=otiles[tc_][:],
                                )
                        return emit

                    for tc_ in range(NCS):
                        for nb in range(2):
                            units.append(unit(tc_, nb))
                    return units

                pending = []  # deferred out-proj units of the previous qblk
                for qblk in range(NSB):
                    nkc = 4 * (qblk + 1)  # causal: k-chunks 0..nkc-1
                    for hp in range(NP):
                        P = [
                            pb.tile([KC, 2 * SB], bf16, tag=f"P{kc}",
                                    name=f"P{kc}_{hp}_{qblk}", bufs=2)
                            for kc in range(nkc)
                        ]
                        pos = [pa.tile([DH + 1, SB], f32, tag=f"ps_av{hh}",
                                       bufs=1, name=f"po{hh}_{hp}_{qblk}")
                               for hh in range(2)]
                        lag = min(LAG, nkc)
                        for slot in range(nkc + lag):
                            if slot < nkc:
                                emit_scores(P, hp, qblk, slot)
                            if hp == 0 and pending:
                                pending.pop(0)()
                            if slot >= lag:
                                emit_av(P, pos, hp, qblk, slot - lag, nkc)
                        emit_norm(pos, hp, qblk)
                    while pending:  # qblk0 window too small for 8 units
                        pending.pop(0)()
                    pending = make_outproj_units(qblk)
                for u in pending:  # final q-block's projection
                    u()

    nc.compile()
    return nc


def _get_program():
    global _compiled
    if _compiled is None:
        _compiled = _build()
    return _compiled


def _shard_inputs(x, Wqkv, bqkv, Wout, bout):
    """Build the 8 per-core input maps (all host-side numpy)."""
    bf = ml_dtypes.bfloat16
    x = np.asarray(x, dtype=np.float32)
    Wqkv = np.asarray(Wqkv, dtype=np.float32)
    bqkv = np.ascontiguousarray(np.asarray(bqkv, dtype=np.float32))
    Wout = np.asarray(Wout, dtype=np.float32)

    Wq = Wqkv[:, 0 * D:1 * D]
    Wk = Wqkv[:, 1 * D:2 * D]
    Wv_full = Wqkv[:, 2 * D:3 * D]
    bq = bqkv[0 * D:1 * D]
    bk = bqkv[1 * D:2 * D]
    bv_full = bqkv[2 * D:3 * D]

    xt_b = [np.ascontiguousarray(
        x[b].T.reshape(D, NSB, SB).transpose(1, 0, 2)
            .reshape(NSB, NDC, KC, SB).astype(bf)) for b in range(B)]

    in_maps = []
    for c in range(NCORES):
        g, i = c // 4, c % 4
        hs = 4 * i * DH           # first head-dim column of my 4 heads
        wqk_c = np.stack([
            np.concatenate(
                [Wq[:, hs + (2 * hp) * DH: hs + (2 * hp + 1) * DH],
                 Wq[:, hs + (2 * hp + 1) * DH: hs + (2 * hp + 2) * DH],
                 Wk[:, hs + (2 * hp) * DH: hs + (2 * hp + 1) * DH],
                 Wk[:, hs + (2 * hp + 1) * DH: hs + (2 * hp + 2) * DH]],
                axis=1).reshape(NDC, KC, 2 * KC)
            for hp in range(NP)])
        bqk_c = np.stack([
            np.stack([
                np.concatenate([bq[hs + (2 * hp) * DH: hs + (2 * hp + 1) * DH],
                                bq[hs + (2 * hp + 1) * DH:
                                   hs + (2 * hp + 2) * DH]]),
                np.concatenate([bk[hs + (2 * hp) * DH: hs + (2 * hp + 1) * DH],
                                bk[hs + (2 * hp + 1) * DH:
                                   hs + (2 * hp + 2) * DH]])])
            for hp in range(NP)])
        wv_c = np.ascontiguousarray(
            Wv_full[:, hs:hs + 4 * DH].reshape(NDC, KC, 2 * KC).astype(bf))
        bv_c = np.ascontiguousarray(bv_full[hs:hs + 4 * DH])
        wout_c = np.ascontiguousarray(
            Wout[hs:hs + 4 * DH].reshape(NP, KC, D).astype(bf))
        in_maps.append({
            "xt": xt_b[g],
            "wqk": np.ascontiguousarray(wqk_c.astype(bf)),
            "wv": wv_c,
            "wout": wout_c,
            "bqk": np.ascontiguousarray(bqk_c.astype(np.float32)),
            "bv": bv_c,
        })
    return in_maps


def run(inputs, trace=False, trace_kwargs=None):
    nc = _get_program()
    in_maps = _shard_inputs(**inputs)
    res = run_bass_kernel_spmd(
        nc, in_maps, list(range(NCORES)), trace=trace,
        **(trace_kwargs or {}),
    )
    bout = np.asarray(inputs["bout"], dtype=np.float32)
    out = np.empty((B, S, D), dtype=np.float32)
    for b in range(B):
        acc = np.zeros((S, D), dtype=np.float32)
        for i in range(4):
            acc += np.asarray(res.results[4 * b + i]["out"],
                              dtype=np.float32).reshape(S, D)
        out[b] = acc + bout
    return out, res


def kernel(**inputs):
    out, _ = run(inputs)
    return out


# revision 12
# speedup vs baseline: 1.0445x; 1.0445x over previous
"""Causal multi-head attention (B=2, S=2048, D=1024, H=16) on 8 trn2 cores.

Sharding: core c handles heads {4i..4i+3} (i = c%4) of batch c//4 only.
All matmul operands are bfloat16 (PSUM accumulates fp32). The output
projection is row-parallel: each core multiplies its own heads' attention
output by the matching 256 rows of Wout, producing a bf16 partial
[2048, 1024] for its batch; the host unshards by summing the four
partials per batch and adding bout. No device collective is needed.

Per core:
  - project host-pretransposed x_b^T [D, S] through the core's Wqkv
    column slice into Q^T/K^T head-pair tiles and V (natural layout with
    a fused ones-column so the AV matmul emits softmax denominators),
  - causal attention per head pair in transposed layout: scores^T = K Q^T
    (row-tiled head pair), exp on ScalarE, causal diagonal-band masks via
    gpsimd affine_select (band kept 128 wide so the predicate iota stays
    exact in bf16), A^T V on PE, normalization straight out of PSUM.

Schedule: the ScalarE exp stream (~73us) is nearly as long as the whole
attention-phase PE work, so the PE stream is issued as
scores(kc) / AV(kc-LAG) interleaved -- the PE then tracks the exp stream
instead of racing ahead into PSUM-bank stalls. The previous q-block's
output projection is threaded into the next block's scores window, and
out tiles are stored as full [128, 1024] rows with one DMA each.
"""

import sys

for _p in ("/opt/trn_rl_repo", "/opt/pypackages"):
    if _p not in sys.path:
        sys.path.insert(0, _p)

import numpy as np
import ml_dtypes

import concourse.bass as bass
import concourse.mybir as mybir
import concourse.tile as tile
from concourse import bacc
from concourse.bass_utils import run_bass_kernel_spmd

B = 2
S = 2048
D = 1024
H = 16
DH = 64
NCORES = 8
SB = 512           # q block (matmul moving dim)
KC = 128           # k chunk (contraction tile)
NSB = S // SB      # 4 q-blocks
NKC = S // KC      # 16 k-chunks
NDC = D // KC      # 8 contraction chunks for the projections
NCS = SB // KC     # 4 k-chunks per sequence block
NP = 2             # head pairs per core
LAG = 3            # AV trails scores by LAG chunks in the PE stream

_compiled = None


def _build():
    f32 = mybir.dt.float32
    bf16 = mybir.dt.bfloat16
    nc = bacc.Bacc(None, target_bir_lowering=False)

    # host-blocked inputs: every [128, N] tile is contiguous in DRAM.
    xt = nc.declare_dram_parameter("xt", [NSB, NDC, KC, SB], bf16, isOutput=False)
    wqk = nc.declare_dram_parameter("wqk", [NP, NDC, KC, 2 * KC], bf16,
                                    isOutput=False)
    wv = nc.declare_dram_parameter("wv", [NDC, KC, 2 * KC], bf16, isOutput=False)
    wout = nc.declare_dram_parameter("wout", [NP, KC, D], bf16, isOutput=False)
    bqk = nc.declare_dram_parameter("bqk", [NP, 2, KC], f32, isOutput=False)
    bv = nc.declare_dram_parameter("bv", [2 * KC], f32, isOutput=False)
    # blocked [16, 128, 1024]: one contiguous 256KB block per token chunk
    out_ext = nc.declare_dram_parameter("out", [NSB * NCS, KC, D], bf16,
                                        isOutput=True)

    with tile.TileContext(nc) as tc:
        with (
            tc.tile_pool(name="qkv", bufs=1) as qkvp,
            tc.tile_pool(name="obuf", bufs=1) as op,
            tc.tile_pool(name="misc", bufs=1) as mp,
            tc.tile_pool(name="evict", bufs=1) as ep,
        ):
            # ---- small constants -----------------------------------------
            bqk_t = [[mp.tile([KC, 1], f32, tag=f"bqk{hp}_{m}",
                              name=f"bqk{hp}_{m}") for m in range(2)]
                     for hp in range(NP)]
            for hp in range(NP):
                for m in range(2):
                    nc.scalar.dma_start(
                        out=bqk_t[hp][m][:],
                        in_=bqk[hp, m].rearrange("(p o) -> p o", o=1),
                    )
            bv_row = mp.tile([1, 2 * KC], f32, tag="bv_row")
            nc.scalar.dma_start(out=bv_row[:], in_=bv.rearrange("(o f) -> o f", o=1))
            bv_bc = mp.tile([KC, 2 * KC], f32, tag="bv_bc")
            nc.gpsimd.partition_broadcast(out_ap=bv_bc[:], in_ap=bv_row[:])

            # Wout row slices for my two head pairs
            wout_t = [mp.tile([KC, D], bf16, tag=f"wo{hp}", name=f"wo{hp}")
                      for hp in range(NP)]

            # ---- persistent activations ----------------------------------
            # pair hp = heads (4i+2hp, 4i+2hp+1) of my batch.
            # QQ[hp]: rows 0:64 = Q^T of the even head, rows 64:128 odd head
            QQ = [[qkvp.tile([KC, SB], bf16, tag=f"QQ{hp}_{s}", name=f"QQ{hp}_{s}")
                   for s in range(NSB)] for hp in range(NP)]
            KK = [[qkvp.tile([KC, SB], bf16, tag=f"KK{hp}_{s}", name=f"KK{hp}_{s}")
                   for s in range(NSB)] for hp in range(NP)]
            # V[hp][s]: [128, 4*2*65]; chunk sc head hh at cols
            # (sc*2+hh)*65 .. +64; col +64 holds 1.0 (denominator trick)
            V = [[qkvp.tile([KC, NCS * 2 * (DH + 1)], bf16, tag=f"V{hp}_{s}",
                            name=f"V{hp}_{s}")
                  for s in range(NSB)] for hp in range(NP)]
            for hp in range(NP):
                for s in range(NSB):
                    vv = V[hp][s][:].rearrange("p (k h c) -> p k h c", h=2,
                                               c=DH + 1)
                    nc.vector.memset(vv[:, :, :, DH], 1.0)
            # O[hp]: rows 0:64 = even head out^T (normalized), 64:128 odd
            O = [op.tile([KC, S], bf16, tag=f"O{hp}", name=f"O{hp}")
                 for hp in range(NP)]

            # ---- phase 1: projections ------------------------------------
            with (
                tc.tile_pool(name="pjw", bufs=1) as wp,
                tc.tile_pool(name="xbuf", bufs=1) as xp,
                tc.tile_pool(name="psum_proj", bufs=1, space="PSUM") as pp,
            ):
                wqk_t = [[wp.tile([KC, 2 * KC], bf16, tag=f"wqk{hp}_{k}",
                                  name=f"wqk{hp}_{k}") for k in range(NDC)]
                         for hp in range(NP)]
                wv_t = [wp.tile([KC, 2 * KC], bf16, tag=f"wv{k}", name=f"wv{k}")
                        for k in range(NDC)]

                # startup-critical DMA order. The first QK chain needs
                # x sblk0 + wqk hp0; keep those OFF scalar (its queue is
                # behind the bqk/bv loads) and interleave x0/wqk0 so the
                # k=0 chunks of both land first. wv is NOT deferred (a
                # deferred wv stalled the PE 8us).
                xtiles = [[None] * NDC for _ in range(NSB)]
                for sblk in range(NSB):
                    for k in range(NDC):
                        xtiles[sblk][k] = xp.tile(
                            [KC, SB], bf16, tag=f"x{sblk}_{k}",
                            name=f"x{sblk}_{k}")
                for k in range(NDC):
                    eng = nc.sync if k % 2 == 0 else nc.gpsimd
                    eng.dma_start(out=xtiles[0][k][:], in_=xt[0, k])
                    eng2 = nc.gpsimd if k % 2 == 0 else nc.sync
                    eng2.dma_start(out=wqk_t[0][k][:], in_=wqk[0, k])
                for k in range(NDC):
                    nc.scalar.dma_start(out=wqk_t[1][k][:], in_=wqk[1, k])
                dma_engs = [nc.sync, nc.gpsimd, nc.scalar]
                _dma_i = [0]

                def dma(out_t, in_t):
                    dma_engs[_dma_i[0] % 3].dma_start(out=out_t, in_=in_t)
                    _dma_i[0] += 1

                for k in range(NDC):
                    dma(wv_t[k][:], wv[k])
                for k in range(NDC):
                    dma(xtiles[1][k][:], xt[1, k])
                for hp in range(NP):
                    dma(wout_t[hp][:], wout[hp])
                for sblk in range(2, NSB):
                    for k in range(NDC):
                        dma(xtiles[sblk][k][:], xt[sblk, k])

                # warm the ScalarE Exp table once the critical DMAs are
                # queued, so the first attention exp starts instantly
                warm = mp.tile([1, 4], f32, tag="warm")
                nc.vector.memset(warm[:], 0.0)
                nc.scalar.activation(warm[:], warm[:],
                                     mybir.ActivationFunctionType.Exp)

                for sblk in range(NSB):
                    xall = xtiles[sblk]
                    for hp in range(NP):
                        # m-chunk 0 -> QQ[hp], 1 -> KK[hp]
                        for m in range(2):
                            ps = pp.tile([KC, SB], f32, tag="ps_qk", bufs=4)
                            for k in range(NDC):
                                nc.tensor.matmul(
                                    ps[:],
                                    wqk_t[hp][k][:, m * KC:(m + 1) * KC],
                                    xall[k][:],
                                    start=(k == 0),
                                    stop=(k == NDC - 1),
                                )
                            dest = (QQ if m == 0 else KK)[hp][sblk]
                            nc.vector.tensor_scalar_add(
                                dest[:], ps[:], bqk_t[hp][m][:],
                            )
                    # V natural: lhsT = x^T chunk; rhs = Wv [128, 256]
                    for sc in range(NCS):
                        ps = pp.tile([KC, 2 * KC], f32, tag="ps_v", bufs=4)
                        for k in range(NDC):
                            nc.tensor.matmul(
                                ps[:],
                                xall[k][:, sc * KC:(sc + 1) * KC],
                                wv_t[k][:],
                                start=(k == 0),
                                stop=(k == NDC - 1),
                            )
                        for hp in range(NP):
                            vslc = V[hp][sblk][:, sc * 2 * (DH + 1):
                                               (sc + 1) * 2 * (DH + 1)]
                            vv = vslc.rearrange("p (h c) -> p h c", c=DH + 1)
                            ps2 = ps[:, hp * KC:(hp + 1) * KC].rearrange(
                                "p (h c) -> p h c", c=DH)
                            bv2 = bv_bc[:, hp * KC:(hp + 1) * KC].rearrange(
                                "p (h c) -> p h c", c=DH)
                            nc.vector.tensor_add(vv[:, :, 0:DH], ps2[:], bv2[:])

            # ---- phase 2: attention + interleaved partial out-proj -------
            with (
                tc.tile_pool(name="pbuf", bufs=1) as pb,
                tc.tile_pool(name="psum_att", bufs=1, space="PSUM") as pa,
            ):
                def emit_scores(P, hp, qblk, kc):
                    d = kc - 4 * qblk
                    # causal: columns < 128*d are fully masked; skip them
                    c0 = KC * max(d, 0)
                    pd3 = P[kc][:].rearrange("p (h f) -> p h f", h=2)
                    for hh in range(2):  # row-tiled head pair
                        r0 = hh * DH
                        # per-hh PSUM tile (1 bank): fine-grained release
                        # keeps the PE only one chunk behind the exp stream
                        ps = pa.tile([KC, SB], f32, tag="ps_s", bufs=2,
                                     name=f"ps_s{qblk}_{hp}_{kc}_{hh}")
                        nc.tensor.matmul(
                            ps[:, c0:SB],
                            KK[hp][kc // 4][r0:r0 + DH,
                                            (kc % 4) * KC:(kc % 4 + 1) * KC],
                            QQ[hp][qblk][r0:r0 + DH, c0:SB],
                            start=True,
                            stop=True,
                        )
                        nc.scalar.activation(
                            pd3[:, hh, c0:SB],
                            ps[:, c0:SB],
                            mybir.ActivationFunctionType.Exp,
                            scale=1.0 / float(np.sqrt(DH)),
                        )
                    if d >= 0:  # diagonal chunk: zero where k > q
                        # only the 128-col diagonal band needs the mask
                        nc.gpsimd.affine_select(
                            out=pd3[:, :, c0:c0 + KC],
                            in_=pd3[:, :, c0:c0 + KC],
                            pattern=[[0, 2], [1, KC]],
                            compare_op=mybir.AluOpType.is_ge,
                            fill=0.0,
                            base=0,
                            channel_multiplier=-1,
                        )

                def emit_av(P, pos, hp, qblk, kc, nkc):
                    d = kc - 4 * qblk
                    c0 = KC * max(d, 0)
                    for hh in range(2):
                        nc.tensor.matmul(
                            pos[hh][:, c0:SB],
                            V[hp][kc // 4][:,
                                ((kc % 4) * 2 + hh) * (DH + 1):
                                ((kc % 4) * 2 + hh + 1) * (DH + 1)],
                            P[kc][:, hh * SB + c0:(hh + 1) * SB],
                            start=(kc == 0),
                            stop=(kc == nkc - 1),
                        )

                def emit_norm(pos, hp, qblk):
                    for hh in range(2):
                        # custom-DVE ops can't read PSUM: evict the
                        # denominator row to SBUF first
                        den0 = ep.tile([1, SB], f32, tag="den0", bufs=2)
                        nc.vector.tensor_copy(den0[:], pos[hh][DH:DH + 1, :])
                        rden = ep.tile([1, SB], f32, tag="rden", bufs=2)
                        rscr = ep.tile([1, SB], f32, tag="rscr", bufs=2)
                        nc.vector.reciprocal_approx_accurate(
                            rden[:], den0[:], rscr[:])
                        rden_bc = ep.tile([DH, SB], f32, tag="rden_bc",
                                          bufs=2)
                        nc.gpsimd.partition_broadcast(
                            out_ap=rden_bc[:], in_ap=rden[:]
                        )
                        r0 = hh * DH
                        nc.vector.tensor_mul(
                            O[hp][r0:r0 + DH, qblk * SB:(qblk + 1) * SB],
                            pos[hh][0:DH, :],
                            rden_bc[:],
                        )

                # out-projection work units for one q-block; emitted lazily
                # so they thread into the NEXT q-block's scores window
                def make_outproj_units(qblk):
                    units = []
                    otiles = {}

                    def unit(tc_, nb):
                        def emit():
                            t0 = qblk * SB + tc_ * KC
                            if nb == 0:
                                otiles[tc_] = ep.tile(
                                    [KC, D], bf16, tag="osb", bufs=2,
                                    name=f"osb{qblk}_{tc_}")
                            pso = pa.tile([KC, SB], f32, tag="ps_o", bufs=2,
                                          name=f"pso{qblk}_{tc_}_{nb}")
                            for hp in range(NP):
                                nc.tensor.matmul(
                                    pso[:],
                                    O[hp][:, t0:t0 + KC],
                                    wout_t[hp][:, nb * SB:(nb + 1) * SB],
                                    start=(hp == 0),
                                    stop=(hp == NP - 1),
                                )
                            nc.vector.tensor_copy(
                                otiles[tc_][:, nb * SB:(nb + 1) * SB], pso[:])
                            if nb == 1:
                                nc.sync.dma_start(
                                    out=out_ext[qblk * NCS + tc_],
                                    in_=otiles[tc_][:],
                                )
                        return emit

                    for tc_ in range(NCS):
                        for nb in range(2):
                            units.append(unit(tc_, nb))
                    return units

                pending = []  # deferred out-proj units of the previous qblk
                for qblk in range(NSB):
                    nkc = 4 * (qblk + 1)  # causal: k-chunks 0..nkc-1
                    P_all = []
                    pos_all = []
                    for hp in range(NP):
                        P_all.append([
                            pb.tile([KC, 2 * SB], bf16, tag=f"P{kc}",
                                    name=f"P{kc}_{hp}_{qblk}", bufs=2)
                            for kc in range(nkc)
                        ])
                        pos_all.append(
                            [pa.tile([DH + 1, SB], f32, tag=f"ps_av{hh}",
                                     bufs=2, name=f"po{hh}_{hp}_{qblk}")
                             for hh in range(2)])
                    # one slot stream across both head pairs: the AV stream
                    # trails scores by LAG chunks and drains across the
                    # hp boundary, so the PE never races ahead of the exp
                    # stream nor idles at window transitions
                    for slot in range(2 * nkc + LAG):
                        if slot < 2 * nkc:
                            hp_s, kc_s = divmod(slot, nkc)
                            emit_scores(P_all[hp_s], hp_s, qblk, kc_s)
                        if pending and slot >= 4:
                            # delayed so the previous block's norm chain
                            # finishes before the first unit's matmul
                            pending.pop(0)()
                        if slot >= LAG:
                            hp_a, kc_a = divmod(slot - LAG, nkc)
                            if hp_a < NP:
                                emit_av(P_all[hp_a], pos_all[hp_a], hp_a,
                                        qblk, kc_a, nkc)
                                if kc_a == nkc - 1:
                                    emit_norm(pos_all[hp_a], hp_a, qblk)
                    while pending:  # qblk0 window too small for 8 units
                        pending.pop(0)()
                    pending = make_outproj_units(qblk)
                for u in pending:  # final q-block's projection
                    u()

    nc.compile()
    return nc


def _get_program():
    global _compiled
    if _compiled is None:
        _compiled = _build()
    return _compiled


def _shard_inputs(x, Wqkv, bqkv, Wout, bout):
    """Build the 8 per-core input maps (all host-side numpy)."""
    bf = ml_dtypes.bfloat16
    x = np.asarray(x, dtype=np.float32)
    Wqkv = np.asarray(Wqkv, dtype=np.float32)
    bqkv = np.ascontiguousarray(np.asarray(bqkv, dtype=np.float32))
    Wout = np.asarray(Wout, dtype=np.float32)

    Wq = Wqkv[:, 0 * D:1 * D]
    Wk = Wqkv[:, 1 * D:2 * D]
    Wv_full = Wqkv[:, 2 * D:3 * D]
    bq = bqkv[0 * D:1 * D]
    bk = bqkv[1 * D:2 * D]
    bv_full = bqkv[2 * D:3 * D]

    xt_b = [np.ascontiguousarray(
        x[b].T.reshape(D, NSB, SB).transpose(1, 0, 2)
            .reshape(NSB, NDC, KC, SB).astype(bf)) for b in range(B)]

    in_maps = []
    for c in range(NCORES):
        g, i = c // 4, c % 4
        hs = 4 * i * DH           # first head-dim column of my 4 heads
        wqk_c = np.stack([
            np.concatenate(
                [Wq[:, hs + (2 * hp) * DH: hs + (2 * hp + 1) * DH],
                 Wq[:, hs + (2 * hp + 1) * DH: hs + (2 * hp + 2) * DH],
                 Wk[:, hs + (2 * hp) * DH: hs + (2 * hp + 1) * DH],
                 Wk[:, hs + (2 * hp + 1) * DH: hs + (2 * hp + 2) * DH]],
                axis=1).reshape(NDC, KC, 2 * KC)
            for hp in range(NP)])
        bqk_c = np.stack([
            np.stack([
                np.concatenate([bq[hs + (2 * hp) * DH: hs + (2 * hp + 1) * DH],
                                bq[hs + (2 * hp + 1) * DH:
                                   hs + (2 * hp + 2) * DH]]),
                np.concatenate([bk[hs + (2 * hp) * DH: hs + (2 * hp + 1) * DH],
                                bk[hs + (2 * hp + 1) * DH:
                                   hs + (2 * hp + 2) * DH]])])
            for hp in range(NP)])
        wv_c = np.ascontiguousarray(
            Wv_full[:, hs:hs + 4 * DH].reshape(NDC, KC, 2 * KC).astype(bf))
        bv_c = np.ascontiguousarray(bv_full[hs:hs + 4 * DH])
        wout_c = np.ascontiguousarray(
            Wout[hs:hs + 4 * DH].reshape(NP, KC, D).astype(bf))
        in_maps.append({
            "xt": xt_b[g],
            "wqk": np.ascontiguousarray(wqk_c.astype(bf)),
            "wv": wv_c,
            "wout": wout_c,
            "bqk": np.ascontiguousarray(bqk_c.astype(np.float32)),
            "bv": bv_c,
        })
    return in_maps


def run(inputs, trace=False, trace_kwargs=None):
    nc = _get_program()
    in_maps = _shard_inputs(**inputs)
    res = run_bass_kernel_spmd(
        nc, in_maps, list(range(NCORES)), trace=trace,
        **(trace_kwargs or {}),
    )
    bout = np.asarray(inputs["bout"], dtype=np.float32)
    out = np.empty((B, S, D), dtype=np.float32)
    for b in range(B):
        acc = np.zeros((S, D), dtype=np.float32)
        for i in range(4):
            acc += np.asarray(res.results[4 * b + i]["out"],
                              dtype=np.float32).reshape(S, D)
        out[b] = acc + bout
    return out, res


def kernel(**inputs):
    out, _ = run(inputs)
    return out


# revision 17
# speedup vs baseline: 1.2322x; 1.1797x over previous
"""Causal multi-head attention (B=2, S=2048, D=1024, H=16) on 8 trn2 cores.

Sharding: core c handles heads {4i..4i+3} (i = c%4) of batch c//4 only.
All matmul operands are bfloat16 (PSUM accumulates fp32). The output
projection is row-parallel: each core multiplies its own heads' attention
output by the matching 256 rows of Wout, producing a bf16 partial
[2048, 1024] for its batch; the host unshards by summing the four
partials per batch and adding bout. No device collective is needed.

Per core:
  - project host-pretransposed x_b^T [D, S] through the core's Wqkv
    column slice into Q^T/K^T head-pair tiles and V (natural layout with
    a fused ones-column so the AV matmul emits softmax denominators),
  - causal attention per head pair in transposed layout: scores^T = K Q^T
    (row-tiled head pair), exp on ScalarE, causal diagonal-band masks via
    gpsimd affine_select (band kept 128 wide so the predicate iota stays
    exact in bf16), A^T V on PE, normalization straight out of PSUM.

Schedule: the ScalarE exp stream (~73us) is nearly as long as the whole
attention-phase PE work, so the PE stream is issued as
scores(kc) / AV(kc-LAG) interleaved -- the PE then tracks the exp stream
instead of racing ahead into PSUM-bank stalls. The previous q-block's
output projection is threaded into the next block's scores window, and
out tiles are stored as full [128, 1024] rows with one DMA each.
"""

import sys

for _p in ("/opt/trn_rl_repo", "/opt/pypackages"):
    if _p not in sys.path:
        sys.path.insert(0, _p)

import numpy as np
import ml_dtypes

import concourse.bass as bass
import concourse.mybir as mybir
import concourse.tile as tile
from concourse import bacc
from concourse.bass_utils import run_bass_kernel_spmd

B = 2
S = 2048
D = 1024
H = 16
DH = 64
NCORES = 8
SB = 512           # q block (matmul moving dim)
KC = 128           # k chunk (contraction tile)
NSB = S // SB      # 4 q-blocks
NKC = S // KC      # 16 k-chunks
NDC = D // KC      # 8 contraction chunks for the projections
NCS = SB // KC     # 4 k-chunks per sequence block
NP = 2             # head pairs per core
LAG = 3            # AV trails scores by LAG chunks in the PE stream

_compiled = None


def _build():
    f32 = mybir.dt.float32
    bf16 = mybir.dt.bfloat16
    nc = bacc.Bacc(None, target_bir_lowering=False)

    # host-blocked inputs: every [128, N] tile is contiguous in DRAM.
    xt = nc.declare_dram_parameter("xt", [NSB, NDC, KC, SB], bf16, isOutput=False)
    wqk = nc.declare_dram_parameter("wqk", [NP, NDC, KC, 2 * KC], bf16,
                                    isOutput=False)
    wv = nc.declare_dram_parameter("wv", [NDC, KC, 2 * KC], bf16, isOutput=False)
    wout = nc.declare_dram_parameter("wout", [NP, KC, D], bf16, isOutput=False)
    bqk = nc.declare_dram_parameter("bqk", [NP, 2, KC], f32, isOutput=False)
    bv = nc.declare_dram_parameter("bv", [2 * KC], f32, isOutput=False)
    # blocked [16, 128, 1024]: one contiguous 256KB block per token chunk
    out_ext = nc.declare_dram_parameter("out", [NSB * NCS, KC, D], bf16,
                                        isOutput=True)

    with tile.TileContext(nc) as tc:
        with (
            tc.tile_pool(name="qkv", bufs=1) as qkvp,
            tc.tile_pool(name="obuf", bufs=1) as op,
            tc.tile_pool(name="misc", bufs=1) as mp,
            tc.tile_pool(name="evict", bufs=1) as ep,
        ):
            # ---- small constants -----------------------------------------
            bqk_t = [[mp.tile([KC, 1], f32, tag=f"bqk{hp}_{m}",
                              name=f"bqk{hp}_{m}") for m in range(2)]
                     for hp in range(NP)]
            for hp in range(NP):
                for m in range(2):
                    nc.scalar.dma_start(
                        out=bqk_t[hp][m][:],
                        in_=bqk[hp, m].rearrange("(p o) -> p o", o=1),
                    )
            bv_row = mp.tile([1, 2 * KC], f32, tag="bv_row")
            nc.scalar.dma_start(out=bv_row[:], in_=bv.rearrange("(o f) -> o f", o=1))
            bv_bc = mp.tile([KC, 2 * KC], f32, tag="bv_bc")
            nc.gpsimd.partition_broadcast(out_ap=bv_bc[:], in_ap=bv_row[:])

            # Wout row slices for my two head pairs
            wout_t = [mp.tile([KC, D], bf16, tag=f"wo{hp}", name=f"wo{hp}")
                      for hp in range(NP)]

            # ---- persistent activations ----------------------------------
            # pair hp = heads (4i+2hp, 4i+2hp+1) of my batch.
            # QQ[hp]: rows 0:64 = Q^T of the even head, rows 64:128 odd head
            QQ = [[qkvp.tile([KC, SB], bf16, tag=f"QQ{hp}_{s}", name=f"QQ{hp}_{s}")
                   for s in range(NSB)] for hp in range(NP)]
            KK = [[qkvp.tile([KC, SB], bf16, tag=f"KK{hp}_{s}", name=f"KK{hp}_{s}")
                   for s in range(NSB)] for hp in range(NP)]
            # V[hp][s]: [128, 4*2*65]; chunk sc head hh at cols
            # (sc*2+hh)*65 .. +64; col +64 holds 1.0 (denominator trick)
            V = [[qkvp.tile([KC, NCS * 2 * (DH + 1)], bf16, tag=f"V{hp}_{s}",
                            name=f"V{hp}_{s}")
                  for s in range(NSB)] for hp in range(NP)]
            for hp in range(NP):
                for s in range(NSB):
                    vv = V[hp][s][:].rearrange("p (k h c) -> p k h c", h=2,
                                               c=DH + 1)
                    nc.vector.memset(vv[:, :, :, DH], 1.0)
            # O[hp]: rows 0:64 = even head out^T (normalized), 64:128 odd
            O = [op.tile([KC, S], bf16, tag=f"O{hp}", name=f"O{hp}")
                 for hp in range(NP)]

            # ---- phase 1: projections ------------------------------------
            with (
                tc.tile_pool(name="pjw", bufs=1) as wp,
                tc.tile_pool(name="xbuf", bufs=1) as xp,
                tc.tile_pool(name="psum_proj", bufs=1, space="PSUM") as pp,
            ):
                wqk_t = [[wp.tile([KC, 2 * KC], bf16, tag=f"wqk{hp}_{k}",
                                  name=f"wqk{hp}_{k}") for k in range(NDC)]
                         for hp in range(NP)]
                wv_t = [wp.tile([KC, 2 * KC], bf16, tag=f"wv{k}", name=f"wv{k}")
                        for k in range(NDC)]

                # startup-critical DMA order. The first QK chain needs
                # x sblk0 + wqk hp0; keep those OFF scalar (its queue is
                # behind the bqk/bv loads) and interleave x0/wqk0 so the
                # k=0 chunks of both land first. wv is NOT deferred (a
                # deferred wv stalled the PE 8us).
                xtiles = [[None] * NDC for _ in range(NSB)]
                for sblk in range(NSB):
                    for k in range(NDC):
                        xtiles[sblk][k] = xp.tile(
                            [KC, SB], bf16, tag=f"x{sblk}_{k}",
                            name=f"x{sblk}_{k}")
                for k in range(NDC):
                    eng = nc.sync if k % 2 == 0 else nc.gpsimd
                    eng.dma_start(out=xtiles[0][k][:], in_=xt[0, k])
                    eng2 = nc.gpsimd if k % 2 == 0 else nc.sync
                    eng2.dma_start(out=wqk_t[0][k][:], in_=wqk[0, k])
                for k in range(NDC):
                    nc.scalar.dma_start(out=wqk_t[1][k][:], in_=wqk[1, k])
                dma_engs = [nc.sync, nc.gpsimd, nc.scalar]
                _dma_i = [0]

                def dma(out_t, in_t):
                    dma_engs[_dma_i[0] % 3].dma_start(out=out_t, in_=in_t)
                    _dma_i[0] += 1

                for k in range(NDC):
                    dma(wv_t[k][:], wv[k])
                for k in range(NDC):
                    dma(xtiles[1][k][:], xt[1, k])
                for hp in range(NP):
                    dma(wout_t[hp][:], wout[hp])
                for sblk in range(2, NSB):
                    for k in range(NDC):
                        dma(xtiles[sblk][k][:], xt[sblk, k])

                # warm the ScalarE Exp table once the critical DMAs are
                # queued, so the first attention exp starts instantly
                warm = mp.tile([1, 4], f32, tag="warm")
                nc.vector.memset(warm[:], 0.0)
                nc.scalar.activation(warm[:], warm[:],
                                     mybir.ActivationFunctionType.Exp)

                def emit_qk(sblk):
                    xall = xtiles[sblk]
                    for hp in range(NP):
                        # m-chunk 0 -> QQ[hp], 1 -> KK[hp]
                        for m in range(2):
                            ps = pp.tile([KC, SB], f32, tag="ps_qk", bufs=4)
                            for k in range(NDC):
                                nc.tensor.matmul(
                                    ps[:],
                                    wqk_t[hp][k][:, m * KC:(m + 1) * KC],
                                    xall[k][:],
                                    start=(k == 0),
                                    stop=(k == NDC - 1),
                                )
                            dest = (QQ if m == 0 else KK)[hp][sblk]
                            nc.vector.tensor_scalar_add(
                                dest[:], ps[:], bqk_t[hp][m][:],
                            )

                def emit_v(sblk):
                    # V natural: lhsT = x^T chunk; rhs = Wv [128, 256]
                    xall = xtiles[sblk]
                    for sc in range(NCS):
                        ps = pp.tile([KC, 2 * KC], f32, tag="ps_v", bufs=4)
                        for k in range(NDC):
                            nc.tensor.matmul(
                                ps[:],
                                xall[k][:, sc * KC:(sc + 1) * KC],
                                wv_t[k][:],
                                start=(k == 0),
                                stop=(k == NDC - 1),
                            )
                        for hp in range(NP):
                            vslc = V[hp][sblk][:, sc * 2 * (DH + 1):
                                               (sc + 1) * 2 * (DH + 1)]
                            vv = vslc.rearrange("p (h c) -> p h c", c=DH + 1)
                            ps2 = ps[:, hp * KC:(hp + 1) * KC].rearrange(
                                "p (h c) -> p h c", c=DH)
                            bv2 = bv_bc[:, hp * KC:(hp + 1) * KC].rearrange(
                                "p (h c) -> p h c", c=DH)
                            nc.vector.tensor_add(vv[:, :, 0:DH], ps2[:], bv2[:])

                # QK of sblk0+1 first: ~18us of PE work that only needs
                # x/wqk, giving the wv (and sblk1 x) DMAs arrival slack
                emit_qk(0)
                emit_qk(1)
                emit_v(0)
                emit_v(1)
                emit_qk(2)
                emit_v(2)
                emit_qk(3)
                emit_v(3)

            # ---- phase 2: attention + interleaved partial out-proj -------
            with (
                tc.tile_pool(name="pbuf", bufs=1) as pb,
                tc.tile_pool(name="psum_att", bufs=1, space="PSUM") as pa,
            ):
                def emit_scores(P, hp, qblk, kc):
                    d = kc - 4 * qblk
                    # causal: columns < 128*d are fully masked; skip them
                    c0 = KC * max(d, 0)
                    # one exp instruction per chunk ([128, 2, f]): a per-hh
                    # split costs +140ns/instr on ScalarE and made the exp
                    # stream the phase pacer
                    ps = pa.tile([KC, 2 * SB], f32, tag="ps_s", bufs=2)
                    for hh in range(2):  # row-tiled head pair
                        r0 = hh * DH
                        nc.tensor.matmul(
                            ps[:, hh * SB + c0:(hh + 1) * SB],
                            KK[hp][kc // 4][r0:r0 + DH,
                                            (kc % 4) * KC:(kc % 4 + 1) * KC],
                            QQ[hp][qblk][r0:r0 + DH, c0:SB],
                            start=True,
                            stop=True,
                        )
                    pd3 = P[kc][:].rearrange("p (h f) -> p h f", h=2)
                    if c0 == 0:
                        nc.scalar.activation(
                            P[kc][:],
                            ps[:],
                            mybir.ActivationFunctionType.Exp,
                            scale=1.0 / float(np.sqrt(DH)),
                        )
                    else:
                        ps3 = ps[:].rearrange("p (h f) -> p h f", h=2)
                        nc.scalar.activation(
                            pd3[:, :, c0:SB],
                            ps3[:, :, c0:SB],
                            mybir.ActivationFunctionType.Exp,
                            scale=1.0 / float(np.sqrt(DH)),
                        )
                    if d >= 0:  # diagonal chunk: zero where k > q
                        # only the 128-col diagonal band needs the mask
                        nc.gpsimd.affine_select(
                            out=pd3[:, :, c0:c0 + KC],
                            in_=pd3[:, :, c0:c0 + KC],
                            pattern=[[0, 2], [1, KC]],
                            compare_op=mybir.AluOpType.is_ge,
                            fill=0.0,
                            base=0,
                            channel_multiplier=-1,
                        )

                def emit_av(P, pos, hp, qblk, kc, nkc):
                    d = kc - 4 * qblk
                    c0 = KC * max(d, 0)
                    for hh in range(2):
                        nc.tensor.matmul(
                            pos[hh][:, c0:SB],
                            V[hp][kc // 4][:,
                                ((kc % 4) * 2 + hh) * (DH + 1):
                                ((kc % 4) * 2 + hh + 1) * (DH + 1)],
                            P[kc][:, hh * SB + c0:(hh + 1) * SB],
                            start=(kc == 0),
                            stop=(kc == nkc - 1),
                        )

                def emit_norm(pos, hp, qblk):
                    # evict the raw accumulators to SBUF immediately: the
                    # two copies free the single-buffered pos banks fast so
                    # the next head pair's AV can start; the reciprocal /
                    # broadcast / multiply then run lazily off the PE path
                    posb = []
                    dens = []
                    for hh in range(2):
                        pb_t = ep.tile([DH, SB], f32, tag=f"posb{hh}",
                                       bufs=2, name=f"posb{hh}_{hp}_{qblk}")
                        nc.vector.tensor_copy(pb_t[:], pos[hh][0:DH, :])
                        # den row into its own base-partition-0 tile: the
                        # custom-DVE reciprocal keys on partition 0
                        dn = ep.tile([1, SB], f32, tag=f"den{hh}", bufs=2,
                                     name=f"den{hh}_{hp}_{qblk}")
                        nc.vector.tensor_copy(dn[:], pos[hh][DH:DH + 1, :])
                        posb.append(pb_t)
                        dens.append(dn)
                    for hh in range(2):
                        rden = ep.tile([1, SB], f32, tag="rden", bufs=2)
                        rscr = ep.tile([1, SB], f32, tag="rscr", bufs=2)
                        nc.vector.reciprocal_approx_accurate(
                            rden[:], dens[hh][:], rscr[:])
                        rden_bc = ep.tile([DH, SB], f32, tag="rden_bc",
                                          bufs=2)
                        nc.gpsimd.partition_broadcast(
                            out_ap=rden_bc[:], in_ap=rden[:]
                        )
                        r0 = hh * DH
                        nc.vector.tensor_mul(
                            O[hp][r0:r0 + DH, qblk * SB:(qblk + 1) * SB],
                            posb[hh][0:DH, :],
                            rden_bc[:],
                        )

                # out-projection work units for one q-block; emitted lazily
                # so they thread into the NEXT q-block's scores window
                def make_outproj_units(qblk):
                    units = []
                    otiles = {}

                    def unit(tc_, nb):
                        def emit():
                            t0 = qblk * SB + tc_ * KC
                            if nb == 0:
                                otiles[tc_] = ep.tile(
                                    [KC, D], bf16, tag="osb", bufs=2,
                                    name=f"osb{qblk}_{tc_}")
                            pso = pa.tile([KC, SB], f32, tag="ps_o", bufs=2,
                                          name=f"pso{qblk}_{tc_}_{nb}")
                            for hp in range(NP):
                                nc.tensor.matmul(
                                    pso[:],
                                    O[hp][:, t0:t0 + KC],
                                    wout_t[hp][:, nb * SB:(nb + 1) * SB],
                                    start=(hp == 0),
                                    stop=(hp == NP - 1),
                                )
                            nc.vector.tensor_copy(
                                otiles[tc_][:, nb * SB:(nb + 1) * SB], pso[:])
                            if nb == 1:
                                nc.sync.dma_start(
                                    out=out_ext[qblk * NCS + tc_],
                                    in_=otiles[tc_][:],
                                )
                        return emit

                    for tc_ in range(NCS):
                        for nb in range(2):
                            units.append(unit(tc_, nb))
                    return units

                pending = []  # deferred out-proj units of the previous qblk
                for qblk in range(NSB):
                    nkc = 4 * (qblk + 1)  # causal: k-chunks 0..nkc-1
                    P_all = []
                    pos_all = []
                    for hp in range(NP):
                        P_all.append([
                            pb.tile([KC, 2 * SB], bf16, tag=f"P{kc}",
                                    name=f"P{kc}_{hp}_{qblk}", bufs=2)
                            for kc in range(nkc)
                        ])
                        pos_all.append(
                            [pa.tile([DH + 1, SB], f32, tag=f"ps_av{hh}",
                                     bufs=1, name=f"po{hh}_{hp}_{qblk}")
                             for hh in range(2)])
                    # one slot stream across both head pairs: the AV stream
                    # trails scores by LAG chunks and drains across the
                    # hp boundary, so the PE never races ahead of the exp
                    # stream nor idles at window transitions
                    for slot in range(2 * nkc + LAG):
                        if slot < 2 * nkc:
                            hp_s, kc_s = divmod(slot, nkc)
                            emit_scores(P_all[hp_s], hp_s, qblk, kc_s)
                        if pending and slot >= 4:
                            # delayed so the previous block's norm chain
                            # finishes before the first unit's matmul
                            pending.pop(0)()
                        if slot >= LAG:
                            hp_a, kc_a = divmod(slot - LAG, nkc)
                            if hp_a < NP:
                                emit_av(P_all[hp_a], pos_all[hp_a], hp_a,
                                        qblk, kc_a, nkc)
                                if kc_a == nkc - 1:
                                    emit_norm(pos_all[hp_a], hp_a, qblk)
                    while pending:  # qblk0 window too small for 8 units
                        pending.pop(0)()
                    pending = make_outproj_units(qblk)
                for u in pending:  # final q-block's projection
                    u()

    nc.compile()
    return nc


def _get_program():
    global _compiled
    if _compiled is None:
        _compiled = _build()
    return _compiled


def _shard_inputs(x, Wqkv, bqkv, Wout, bout):
    """Build the 8 per-core input maps (all host-side numpy)."""
    bf = ml_dtypes.bfloat16
    x = np.asarray(x, dtype=np.float32)
    Wqkv = np.asarray(Wqkv, dtype=np.float32)
    bqkv = np.ascontiguousarray(np.asarray(bqkv, dtype=np.float32))
    Wout = np.asarray(Wout, dtype=np.float32)

    Wq = Wqkv[:, 0 * D:1 * D]
    Wk = Wqkv[:, 1 * D:2 * D]
    Wv_full = Wqkv[:, 2 * D:3 * D]
    bq = bqkv[0 * D:1 * D]
    bk = bqkv[1 * D:2 * D]
    bv_full = bqkv[2 * D:3 * D]

    xt_b = [np.ascontiguousarray(
        x[b].T.reshape(D, NSB, SB).transpose(1, 0, 2)
            .reshape(NSB, NDC, KC, SB).astype(bf)) for b in range(B)]

    in_maps = []
    for c in range(NCORES):
        g, i = c // 4, c % 4
        hs = 4 * i * DH           # first head-dim column of my 4 heads
        wqk_c = np.stack([
            np.concatenate(
                [Wq[:, hs + (2 * hp) * DH: hs + (2 * hp + 1) * DH],
                 Wq[:, hs + (2 * hp + 1) * DH: hs + (2 * hp + 2) * DH],
                 Wk[:, hs + (2 * hp) * DH: hs + (2 * hp + 1) * DH],
                 Wk[:, hs + (2 * hp + 1) * DH: hs + (2 * hp + 2) * DH]],
                axis=1).reshape(NDC, KC, 2 * KC)
            for hp in range(NP)])
        bqk_c = np.stack([
            np.stack([
                np.concatenate([bq[hs + (2 * hp) * DH: hs + (2 * hp + 1) * DH],
                                bq[hs + (2 * hp + 1) * DH:
                                   hs + (2 * hp + 2) * DH]]),
                np.concatenate([bk[hs + (2 * hp) * DH: hs + (2 * hp + 1) * DH],
                                bk[hs + (2 * hp + 1) * DH:
                                   hs + (2 * hp + 2) * DH]])])
            for hp in range(NP)])
        wv_c = np.ascontiguousarray(
            Wv_full[:, hs:hs + 4 * DH].reshape(NDC, KC, 2 * KC).astype(bf))
        bv_c = np.ascontiguousarray(bv_full[hs:hs + 4 * DH])
        wout_c = np.ascontiguousarray(
            Wout[hs:hs + 4 * DH].reshape(NP, KC, D).astype(bf))
        in_maps.append({
            "xt": xt_b[g],
            "wqk": np.ascontiguousarray(wqk_c.astype(bf)),
            "wv": wv_c,
            "wout": wout_c,
            "bqk": np.ascontiguousarray(bqk_c.astype(np.float32)),
            "bv": bv_c,
        })
    return in_maps


def run(inputs, trace=False, trace_kwargs=None):
    nc = _get_program()
    in_maps = _shard_inputs(**inputs)
    res = run_bass_kernel_spmd(
        nc, in_maps, list(range(NCORES)), trace=trace,
        **(trace_kwargs or {}),
    )
    bout = np.asarray(inputs["bout"], dtype=np.float32)
    out = np.empty((B, S, D), dtype=np.float32)
    for b in range(B):
        acc = np.zeros((S, D), dtype=np.float32)
        for i in range(4):
            acc += np.asarray(res.results[4 * b + i]["out"],
                              dtype=np.float32).reshape(S, D)
        out[b] = acc + bout
    return out, res


def kernel(**inputs):
    out, _ = run(inputs)
    return out


# revision 20
# speedup vs baseline: 1.2538x; 1.0176x over previous
"""Causal multi-head attention (B=2, S=2048, D=1024, H=16) on 8 trn2 cores.

Sharding: core c handles heads {4i..4i+3} (i = c%4) of batch c//4 only.
All matmul operands are bfloat16 (PSUM accumulates fp32). The output
projection is row-parallel: each core multiplies its own heads' attention
output by the matching 256 rows of Wout, producing a bf16 partial
[2048, 1024] for its batch; the host unshards by summing the four
partials per batch and adding bout. No device collective is needed.

Per core:
  - project host-pretransposed x_b^T [D, S] through the core's Wqkv
    column slice into Q^T/K^T head-pair tiles and V (natural layout with
    a fused ones-column so the AV matmul emits softmax denominators),
  - causal attention per head pair in transposed layout: scores^T = K Q^T
    (row-tiled head pair), exp on ScalarE, causal diagonal-band masks via
    gpsimd affine_select, A^T V on PE, normalization via fast reciprocal.

Schedule notes (all measured on hw traces):
  - the TRN2 PE has a p-state ramp (0.65/1.2/2.4 GHz, max only after
    ~3us continuously busy), so PE gaps cost ~3x their length; the whole
    phase-2 stream is emitted as one global scores/AV slot queue where
    AV trails scores by LAG chunks and drains across head-pair and
    q-block boundaries,
  - the ScalarE exp stream (~74us) is within a few % of the attention
    PE work; one exp per chunk ([128, 2, f]) -- per-hh splits add
    +140ns/instr and make exp the pacer,
  - AV accumulators are evicted to SBUF immediately (copies) so the
    single-buffered pos PSUM banks free fast; the reciprocal/broadcast/
    multiply run lazily off the PE critical path. Custom DVE ops key on
    partition 0, so the denominator row gets its own [1, SB] tile,
  - phase-1 inputs arrive via ~12 coarse DMAs (contiguous 0.5-1MB
    blocks into wide SBUF tiles): 56 small DMAs were issue-rate bound
    (~640ns descriptor-gen each) and starved the first QK chains.
"""

import sys

for _p in ("/opt/trn_rl_repo", "/opt/pypackages"):
    if _p not in sys.path:
        sys.path.insert(0, _p)

import numpy as np
import ml_dtypes

import concourse.bass as bass
import concourse.mybir as mybir
import concourse.tile as tile
from concourse import bacc
from concourse.bass_utils import run_bass_kernel_spmd

B = 2
S = 2048
D = 1024
H = 16
DH = 64
NCORES = 8
SB = 512           # q block (matmul moving dim)
KC = 128           # k chunk (contraction tile)
NSB = S // SB      # 4 q-blocks
NKC = S // KC      # 16 k-chunks
NDC = D // KC      # 8 contraction chunks for the projections
NCS = SB // KC     # 4 k-chunks per sequence block
NP = 2             # head pairs per core
LAG = 3            # AV trails scores by LAG chunks in the PE stream

_compiled = None


def _build():
    f32 = mybir.dt.float32
    bf16 = mybir.dt.bfloat16
    nc = bacc.Bacc(None, target_bir_lowering=False)

    # host-blocked inputs: every [128, N] tile is contiguous in DRAM.
    xt = nc.declare_dram_parameter("xt", [NSB, NDC, KC, SB], bf16, isOutput=False)
    wqk = nc.declare_dram_parameter("wqk", [NP, NDC, KC, 2 * KC], bf16,
                                    isOutput=False)
    wv = nc.declare_dram_parameter("wv", [NDC, KC, 2 * KC], bf16, isOutput=False)
    wout = nc.declare_dram_parameter("wout", [NP, KC, D], bf16, isOutput=False)
    bqk = nc.declare_dram_parameter("bqk", [NP, 2, KC], f32, isOutput=False)
    bv = nc.declare_dram_parameter("bv", [2 * KC], f32, isOutput=False)
    # blocked [16, 128, 1024]: one contiguous 256KB block per token chunk
    out_ext = nc.declare_dram_parameter("out", [NSB * NCS, KC, D], bf16,
                                        isOutput=True)

    with tile.TileContext(nc) as tc:
        with (
            tc.tile_pool(name="qkv", bufs=1) as qkvp,
            tc.tile_pool(name="obuf", bufs=1) as op,
            tc.tile_pool(name="misc", bufs=1) as mp,
            tc.tile_pool(name="evict", bufs=1) as ep,
        ):
            # ---- small constants -----------------------------------------
            bqk_t = [[mp.tile([KC, 1], f32, tag=f"bqk{hp}_{m}",
                              name=f"bqk{hp}_{m}") for m in range(2)]
                     for hp in range(NP)]
            for hp in range(NP):
                for m in range(2):
                    nc.scalar.dma_start(
                        out=bqk_t[hp][m][:],
                        in_=bqk[hp, m].rearrange("(p o) -> p o", o=1),
                    )
            bv_row = mp.tile([1, 2 * KC], f32, tag="bv_row")
            nc.scalar.dma_start(out=bv_row[:], in_=bv.rearrange("(o f) -> o f", o=1))
            bv_bc = mp.tile([KC, 2 * KC], f32, tag="bv_bc")
            nc.gpsimd.partition_broadcast(out_ap=bv_bc[:], in_ap=bv_row[:])

            # Wout row slices for my two head pairs
            wout_t = [mp.tile([KC, D], bf16, tag=f"wo{hp}", name=f"wo{hp}")
                      for hp in range(NP)]

            # ---- persistent activations ----------------------------------
            # pair hp = heads (4i+2hp, 4i+2hp+1) of my batch.
            # QQ[hp]: rows 0:64 = Q^T of the even head, rows 64:128 odd head
            QQ = [[qkvp.tile([KC, SB], bf16, tag=f"QQ{hp}_{s}", name=f"QQ{hp}_{s}")
                   for s in range(NSB)] for hp in range(NP)]
            KK = [[qkvp.tile([KC, SB], bf16, tag=f"KK{hp}_{s}", name=f"KK{hp}_{s}")
                   for s in range(NSB)] for hp in range(NP)]
            # V[hp][s]: [128, 4*2*65]; chunk sc head hh at cols
            # (sc*2+hh)*65 .. +64; col +64 holds 1.0 (denominator trick)
            V = [[qkvp.tile([KC, NCS * 2 * (DH + 1)], bf16, tag=f"V{hp}_{s}",
                            name=f"V{hp}_{s}")
                  for s in range(NSB)] for hp in range(NP)]
            for hp in range(NP):
                for s in range(NSB):
                    vv = V[hp][s][:].rearrange("p (k h c) -> p k h c", h=2,
                                               c=DH + 1)
                    nc.vector.memset(vv[:, :, :, DH], 1.0)
            # O[hp]: rows 0:64 = even head out^T (normalized), 64:128 odd
            O = [op.tile([KC, S], bf16, tag=f"O{hp}", name=f"O{hp}")
                 for hp in range(NP)]

            # ---- phase 1: projections ------------------------------------
            with (
                tc.tile_pool(name="pjw", bufs=1) as wp,
                tc.tile_pool(name="xbuf", bufs=1) as xp,
                tc.tile_pool(name="psum_proj", bufs=1, space="PSUM") as pp,
            ):
                # wide tiles, one coarse DMA per contiguous DRAM block
                wqkt = [wp.tile([KC, NDC * 2 * KC], bf16, tag=f"wqk{hp}",
                                name=f"wqk{hp}") for hp in range(NP)]
                wvt = wp.tile([KC, NDC * 2 * KC], bf16, tag="wvt")
                xsb = [xp.tile([KC, NDC * SB], bf16, tag=f"x{s}", name=f"x{s}")
                       for s in range(NSB)]

                def blk(t, n):
                    # partition dim stays first on the SBUF side
                    return t[:].rearrange("p (k f) -> p k f", k=n)

                def pmaj(dram):
                    return dram.rearrange("k p f -> p k f")

                nc.sync.dma_start(out=blk(xsb[0], NDC), in_=pmaj(xt[0]))
                nc.gpsimd.dma_start(out=blk(wqkt[0], NDC), in_=pmaj(wqk[0]))
                nc.gpsimd.dma_start(out=blk(wqkt[1], NDC), in_=pmaj(wqk[1]))
                nc.sync.dma_start(out=blk(wvt, NDC), in_=pmaj(wv[0:NDC]))
                nc.gpsimd.dma_start(out=blk(xsb[1], NDC), in_=pmaj(xt[1]))
                nc.scalar.dma_start(out=wout_t[0][:], in_=wout[0])
                nc.scalar.dma_start(out=wout_t[1][:], in_=wout[1])
                nc.sync.dma_start(out=blk(xsb[2], NDC), in_=pmaj(xt[2]))
                nc.gpsimd.dma_start(out=blk(xsb[3], NDC), in_=pmaj(xt[3]))

                # warm the ScalarE Exp table so the first attention exp
                # starts instantly
                warm = mp.tile([1, 4], f32, tag="warm")
                nc.vector.memset(warm[:], 0.0)
                nc.scalar.activation(warm[:], warm[:],
                                     mybir.ActivationFunctionType.Exp)

                def emit_qk(sblk):
                    for hp in range(NP):
                        # m-chunk 0 -> QQ[hp], 1 -> KK[hp]
                        for m in range(2):
                            ps = pp.tile([KC, SB], f32, tag="ps_qk", bufs=4)
                            for k in range(NDC):
                                nc.tensor.matmul(
                                    ps[:],
                                    wqkt[hp][:, k * 2 * KC + m * KC:
                                             k * 2 * KC + (m + 1) * KC],
                                    xsb[sblk][:, k * SB:(k + 1) * SB],
                                    start=(k == 0),
                                    stop=(k == NDC - 1),
                                )
                            dest = (QQ if m == 0 else KK)[hp][sblk]
                            nc.vector.tensor_scalar_add(
                                dest[:], ps[:], bqk_t[hp][m][:],
                            )

                def emit_v(sblk):
                    # V natural: lhsT = x^T chunk; rhs = Wv [128, 256]
                    for sc in range(NCS):
                        ps = pp.tile([KC, 2 * KC], f32, tag="ps_v", bufs=4)
                        for k in range(NDC):
                            nc.tensor.matmul(
                                ps[:],
                                xsb[sblk][:, k * SB + sc * KC:
                                          k * SB + (sc + 1) * KC],
                                wvt[:, k * 2 * KC:(k + 1) * 2 * KC],
                                start=(k == 0),
                                stop=(k == NDC - 1),
                            )
                        for hp in range(NP):
                            vslc = V[hp][sblk][:, sc * 2 * (DH + 1):
                                               (sc + 1) * 2 * (DH + 1)]
                            vv = vslc.rearrange("p (h c) -> p h c", c=DH + 1)
                            ps2 = ps[:, hp * KC:(hp + 1) * KC].rearrange(
                                "p (h c) -> p h c", c=DH)
                            bv2 = bv_bc[:, hp * KC:(hp + 1) * KC].rearrange(
                                "p (h c) -> p h c", c=DH)
                            nc.vector.tensor_add(vv[:, :, 0:DH], ps2[:], bv2[:])

                # QK of sblk0+1 first: ~18us of PE work that only needs
                # x/wqk, giving the wv / sblk1 x DMAs arrival slack
                emit_qk(0)
                emit_qk(1)
                emit_v(0)
                emit_v(1)
                emit_qk(2)
                emit_v(2)
                emit_qk(3)
                emit_v(3)

            # ---- phase 2: attention + interleaved partial out-proj -------
            with (
                tc.tile_pool(name="pbuf", bufs=1) as pb,
                tc.tile_pool(name="psum_att", bufs=1, space="PSUM") as pa,
            ):
                def make_scores(P, hp, qblk, kc):
                    def emit():
                        d = kc - 4 * qblk
                        # causal: cols < 128*d are fully masked; skip them
                        c0 = KC * max(d, 0)
                        ps = pa.tile([KC, 2 * SB], f32, tag="ps_s", bufs=2)
                        for hh in range(2):  # row-tiled head pair
                            r0 = hh * DH
                            nc.tensor.matmul(
                                ps[:, hh * SB + c0:(hh + 1) * SB],
                                KK[hp][kc // 4][r0:r0 + DH,
                                                (kc % 4) * KC:
                                                (kc % 4 + 1) * KC],
                                QQ[hp][qblk][r0:r0 + DH, c0:SB],
                                start=True,
                                stop=True,
                            )
                        pd3 = P[kc][:].rearrange("p (h f) -> p h f", h=2)
                        if c0 == 0:
                            nc.scalar.activation(
                                P[kc][:],
                                ps[:],
                                mybir.ActivationFunctionType.Exp,
                                scale=1.0 / float(np.sqrt(DH)),
                            )
                        else:
                            ps3 = ps[:].rearrange("p (h f) -> p h f", h=2)
                            nc.scalar.activation(
                                pd3[:, :, c0:SB],
                                ps3[:, :, c0:SB],
                                mybir.ActivationFunctionType.Exp,
                                scale=1.0 / float(np.sqrt(DH)),
                            )
                        if d >= 0:  # diagonal chunk: zero where k > q
                            nc.gpsimd.affine_select(
                                out=pd3[:, :, c0:c0 + KC],
                                in_=pd3[:, :, c0:c0 + KC],
                                pattern=[[0, 2], [1, KC]],
                                compare_op=mybir.AluOpType.is_ge,
                                fill=0.0,
                                base=0,
                                channel_multiplier=-1,
                            )
                    return emit

                def make_av(P, pos, hp, qblk, kc, nkc):
                    def emit():
                        d = kc - 4 * qblk
                        c0 = KC * max(d, 0)
                        for hh in range(2):
                            nc.tensor.matmul(
                                pos[hh][:, c0:SB],
                                V[hp][kc // 4][:,
                                    ((kc % 4) * 2 + hh) * (DH + 1):
                                    ((kc % 4) * 2 + hh + 1) * (DH + 1)],
                                P[kc][:, hh * SB + c0:(hh + 1) * SB],
                                start=(kc == 0),
                                stop=(kc == nkc - 1),
                            )
                    return emit

                def make_norm(pos, hp, qblk):
                    def emit():
                        # den row first (own base-partition-0 tile: custom
                        # DVE ops key on partition 0), then its reciprocal,
                        # then the bulk eviction -- pos banks free after the
                        # copies; recip/broadcast/mult run off the PE path
                        dens, rdens, posb = [], [], []
                        for hh in range(2):
                            dn = ep.tile([1, SB], f32, tag=f"den{hh}",
                                         bufs=2, name=f"den{hh}_{hp}_{qblk}")
                            nc.vector.tensor_copy(dn[:], pos[hh][DH:DH + 1, :])
                            dens.append(dn)
                            rden = ep.tile([1, SB], f32, tag=f"rden{hh}",
                                           bufs=2, name=f"rden{hh}_{hp}_{qblk}")
                            nc.vector.reciprocal_approx_fast(
                                out=rden[:], in_=dn[:])
                            rdens.append(rden)
                        for hh in range(2):
                            pb_t = ep.tile([DH, SB], f32, tag=f"posb{hh}",
                                           bufs=2, name=f"posb{hh}_{hp}_{qblk}")
                            nc.vector.tensor_copy(pb_t[:], pos[hh][0:DH, :])
                            posb.append(pb_t)
                        for hh in range(2):
                            rden_bc = ep.tile([DH, SB], f32, tag="rden_bc",
                                              bufs=2)
                            nc.gpsimd.partition_broadcast(
                                out_ap=rden_bc[:], in_ap=rdens[hh][:])
                            r0 = hh * DH
                            nc.vector.tensor_mul(
                                O[hp][r0:r0 + DH, qblk * SB:(qblk + 1) * SB],
                                posb[hh][:],
                                rden_bc[:],
                            )
                    return emit

                def make_outproj_units(qblk):
                    units = []
                    otiles = {}

                    def unit(tc_, nb):
                        def emit():
                            t0 = qblk * SB + tc_ * KC
                            if nb == 0:
                                otiles[tc_] = ep.tile(
                                    [KC, D], bf16, tag="osb", bufs=2,
                                    name=f"osb{qblk}_{tc_}")
                            pso = pa.tile([KC, SB], f32, tag="ps_o", bufs=2,
                                          name=f"pso{qblk}_{tc_}_{nb}")
                            for hp in range(NP):
                                nc.tensor.matmul(
                                    pso[:],
                                    O[hp][:, t0:t0 + KC],
                                    wout_t[hp][:, nb * SB:(nb + 1) * SB],
                                    start=(hp == 0),
                                    stop=(hp == NP - 1),
                                )
                            nc.vector.tensor_copy(
                                otiles[tc_][:, nb * SB:(nb + 1) * SB], pso[:])
                            if nb == 1:
                                nc.sync.dma_start(
                                    out=out_ext[qblk * NCS + tc_],
                                    in_=otiles[tc_][:],
                                )
                        return emit

                    for tc_ in range(NCS):
                        for nb in range(2):
                            units.append(unit(tc_, nb))
                    return units

                # one global slot stream: scores lead, AV trails by LAG
                # chunks and drains across head-pair and q-block boundaries
                score_emits = []
                av_emits = []
                qblk_of_slot = []
                for qblk in range(NSB):
                    nkc = 4 * (qblk + 1)
                    for hp in range(NP):
                        P = [pb.tile([KC, 2 * SB], bf16, tag=f"P{kc}",
                                     name=f"P{kc}_{hp}_{qblk}", bufs=2)
                             for kc in range(nkc)]
                        pos = [pa.tile([DH + 1, SB], f32, tag=f"ps_av{hh}",
                                       bufs=1, name=f"po{hh}_{hp}_{qblk}")
                               for hh in range(2)]
                        norm = make_norm(pos, hp, qblk)
                        for kc in range(nkc):
                            score_emits.append(make_scores(P, hp, qblk, kc))
                            av = make_av(P, pos, hp, qblk, kc, nkc)
                            if kc == nkc - 1:
                                av_emits.append(
                                    (lambda a, n: lambda: (a(), n()))(av, norm))
                            else:
                                av_emits.append(av)
                            qblk_of_slot.append(qblk)

                n_slots = len(score_emits)
                pending = []
                cur_qblk = 0
                slot_in_qblk = 0
                for slot in range(n_slots + LAG):
                    if slot < n_slots:
                        if qblk_of_slot[slot] != cur_qblk:
                            while pending:
                                pending.pop(0)()
                            cur_qblk = qblk_of_slot[slot]
                            slot_in_qblk = 0
                            pending = make_outproj_units(cur_qblk - 1)
                        score_emits[slot]()
                        if pending and slot_in_qblk >= 4:
                            pending.pop(0)()
                        slot_in_qblk += 1
                    if slot >= LAG:
                        av_emits[slot - LAG]()
                while pending:
                    pending.pop(0)()
                for u in make_outproj_units(NSB - 1):
                    u()

    nc.compile()
    return nc


def _get_program():
    global _compiled
    if _compiled is None:
        _compiled = _build()
    return _compiled


def _shard_inputs(x, Wqkv, bqkv, Wout, bout):
    """Build the 8 per-core input maps (all host-side numpy)."""
    bf = ml_dtypes.bfloat16
    x = np.asarray(x, dtype=np.float32)
    Wqkv = np.asarray(Wqkv, dtype=np.float32)
    bqkv = np.ascontiguousarray(np.asarray(bqkv, dtype=np.float32))
    Wout = np.asarray(Wout, dtype=np.float32)

    Wq = Wqkv[:, 0 * D:1 * D]
    Wk = Wqkv[:, 1 * D:2 * D]
    Wv_full = Wqkv[:, 2 * D:3 * D]
    bq = bqkv[0 * D:1 * D]
    bk = bqkv[1 * D:2 * D]
    bv_full = bqkv[2 * D:3 * D]

    xt_b = [np.ascontiguousarray(
        x[b].T.reshape(D, NSB, SB).transpose(1, 0, 2)
            .reshape(NSB, NDC, KC, SB).astype(bf)) for b in range(B)]

    in_maps = []
    for c in range(NCORES):
        g, i = c // 4, c % 4
        hs = 4 * i * DH           # first head-dim column of my 4 heads
        wqk_c = np.stack([
            np.concatenate(
                [Wq[:, hs + (2 * hp) * DH: hs + (2 * hp + 1) * DH],
                 Wq[:, hs + (2 * hp + 1) * DH: hs + (2 * hp + 2) * DH],
                 Wk[:, hs + (2 * hp) * DH: hs + (2 * hp + 1) * DH],
                 Wk[:, hs + (2 * hp + 1) * DH: hs + (2 * hp + 2) * DH]],
                axis=1).reshape(NDC, KC, 2 * KC)
            for hp in range(NP)])
        bqk_c = np.stack([
            np.stack([
                np.concatenate([bq[hs + (2 * hp) * DH: hs + (2 * hp + 1) * DH],
                                bq[hs + (2 * hp + 1) * DH:
                                   hs + (2 * hp + 2) * DH]]),
                np.concatenate([bk[hs + (2 * hp) * DH: hs + (2 * hp + 1) * DH],
                                bk[hs + (2 * hp + 1) * DH:
                                   hs + (2 * hp + 2) * DH]])])
            for hp in range(NP)])
        wv_c = np.ascontiguousarray(
            Wv_full[:, hs:hs + 4 * DH].reshape(NDC, KC, 2 * KC).astype(bf))
        bv_c = np.ascontiguousarray(bv_full[hs:hs + 4 * DH])
        wout_c = np.ascontiguousarray(
            Wout[hs:hs + 4 * DH].reshape(NP, KC, D).astype(bf))
        in_maps.append({
            "xt": xt_b[g],
            "wqk": np.ascontiguousarray(wqk_c.astype(bf)),
            "wv": wv_c,
            "wout": wout_c,
            "bqk": np.ascontiguousarray(bqk_c.astype(np.float32)),
            "bv": bv_c,
        })
    return in_maps


def run(inputs, trace=False, trace_kwargs=None):
    nc = _get_program()
    in_maps = _shard_inputs(**inputs)
    res = run_bass_kernel_spmd(
        nc, in_maps, list(range(NCORES)), trace=trace,
        **(trace_kwargs or {}),
    )
    bout = np.asarray(inputs["bout"], dtype=np.float32)
    out = np.empty((B, S, D), dtype=np.float32)
    for b in range(B):
        acc = np.zeros((S, D), dtype=np.float32)
        for i in range(4):
            acc += np.asarray(res.results[4 * b + i]["out"],
                              dtype=np.float32).reshape(S, D)
        out[b] = acc + bout
    return out, res


def kernel(**inputs):
    out, _ = run(inputs)
    return out


# revision 27
# speedup vs baseline: 1.2665x; 1.0101x over previous
"""Causal multi-head attention (B=2, S=2048, D=1024, H=16) on 8 trn2 cores.

Sharding: core c handles heads {4i..4i+3} (i = c%4) of batch c//4 only.
All matmul operands are bfloat16 (PSUM accumulates fp32). The output
projection is row-parallel: each core multiplies its own heads' attention
output by the matching 256 rows of Wout, producing a bf16 partial
[2048, 1024] for its batch; the host unshards by summing the four
partials per batch and adding bout. No device collective is needed.

Per core:
  - project host-pretransposed x_b^T [D, S] through the core's Wqkv
    column slice into Q^T/K^T head-pair tiles and V (natural layout with
    a fused ones-column so the AV matmul emits softmax denominators),
  - causal attention per head pair in transposed layout: scores^T = K Q^T
    (row-tiled head pair), exp on ScalarE, causal diagonal-band masks via
    gpsimd affine_select, A^T V on PE, normalization via fast reciprocal.

Schedule notes (all measured on hw traces):
  - the TRN2 PE has a p-state ramp (0.65/1.2/2.4 GHz, max only after
    ~3us continuously busy), so PE gaps cost ~3x their length; the whole
    phase-2 stream is emitted as one global scores/AV slot queue where
    AV trails scores by LAG chunks and drains across head-pair and
    q-block boundaries,
  - the ScalarE exp stream (~74us) is within a few % of the attention
    PE work; one exp per chunk ([128, 2, f]) -- per-hh splits add
    +140ns/instr and make exp the pacer,
  - AV accumulators are evicted to SBUF immediately (copies) so the
    single-buffered pos PSUM banks free fast; the reciprocal/broadcast/
    multiply run lazily off the PE critical path. Custom DVE ops key on
    partition 0, so the denominator row gets its own [1, SB] tile,
  - phase-1 inputs arrive via ~12 coarse DMAs (contiguous 0.5-1MB
    blocks into wide SBUF tiles): 56 small DMAs were issue-rate bound
    (~640ns descriptor-gen each) and starved the first QK chains.
"""

import sys

for _p in ("/opt/trn_rl_repo", "/opt/pypackages"):
    if _p not in sys.path:
        sys.path.insert(0, _p)

import numpy as np
import ml_dtypes

import concourse.bass as bass
import concourse.mybir as mybir
import concourse.tile as tile
from concourse import bacc
from concourse.bass_utils import run_bass_kernel_spmd

B = 2
S = 2048
D = 1024
H = 16
DH = 64
NCORES = 8
SB = 512           # q block (matmul moving dim)
KC = 128           # k chunk (contraction tile)
NSB = S // SB      # 4 q-blocks
NKC = S // KC      # 16 k-chunks
NDC = D // KC      # 8 contraction chunks for the projections
NCS = SB // KC     # 4 k-chunks per sequence block
NP = 2             # head pairs per core
LAG = 4            # AV trails scores by LAG chunks in the PE stream

_compiled = None


def _build():
    f32 = mybir.dt.float32
    bf16 = mybir.dt.bfloat16
    nc = bacc.Bacc(None, target_bir_lowering=False)

    # host-preblocked inputs, p-major so each DMA is contiguous on BOTH sides:
    # xt[s, half] is a [128, 2048] block (d-chunks 4h..4h+3 side by side)
    xt = nc.declare_dram_parameter("xt", [NSB, 2, KC, NDC // 2 * SB], bf16,
                                   isOutput=False)
    wqk = nc.declare_dram_parameter("wqk", [NP, KC, NDC * 2 * KC], bf16,
                                    isOutput=False)
    wv = nc.declare_dram_parameter("wv", [KC, NDC * 2 * KC], bf16, isOutput=False)
    wout = nc.declare_dram_parameter("wout", [NP, KC, D], bf16, isOutput=False)
    bqk = nc.declare_dram_parameter("bqk", [NP, 2, KC], f32, isOutput=False)
    bv = nc.declare_dram_parameter("bv", [2 * KC], f32, isOutput=False)
    # blocked [16, 128, 1024]: one contiguous 256KB block per token chunk
    out_ext = nc.declare_dram_parameter("out", [NSB * NCS, KC, D], bf16,
                                        isOutput=True)

    with tile.TileContext(nc) as tc:
        with (
            tc.tile_pool(name="qkv", bufs=1) as qkvp,
            tc.tile_pool(name="obuf", bufs=1) as op,
            tc.tile_pool(name="misc", bufs=1) as mp,
            tc.tile_pool(name="evict", bufs=1) as ep,
        ):
            # ---- small constants -----------------------------------------
            bqk_t = [[mp.tile([KC, 1], f32, tag=f"bqk{hp}_{m}",
                              name=f"bqk{hp}_{m}") for m in range(2)]
                     for hp in range(NP)]
            for hp in range(NP):
                for m in range(2):
                    nc.scalar.dma_start(
                        out=bqk_t[hp][m][:],
                        in_=bqk[hp, m].rearrange("(p o) -> p o", o=1),
                    )
            bv_row = mp.tile([1, 2 * KC], f32, tag="bv_row")
            nc.scalar.dma_start(out=bv_row[:], in_=bv.rearrange("(o f) -> o f", o=1))
            bv_bc = mp.tile([KC, 2 * KC], f32, tag="bv_bc")
            nc.gpsimd.partition_broadcast(out_ap=bv_bc[:], in_ap=bv_row[:])

            # Wout row slices for my two head pairs
            wout_t = [mp.tile([KC, D], bf16, tag=f"wo{hp}", name=f"wo{hp}")
                      for hp in range(NP)]

            # ---- persistent activations ----------------------------------
            # pair hp = heads (4i+2hp, 4i+2hp+1) of my batch.
            # QQ[hp]: rows 0:64 = Q^T of the even head, rows 64:128 odd head
            QQ = [[qkvp.tile([KC, SB], bf16, tag=f"QQ{hp}_{s}", name=f"QQ{hp}_{s}")
                   for s in range(NSB)] for hp in range(NP)]
            KK = [[qkvp.tile([KC, SB], bf16, tag=f"KK{hp}_{s}", name=f"KK{hp}_{s}")
                   for s in range(NSB)] for hp in range(NP)]
            # V[hp][s]: [128, 4*2*65]; chunk sc head hh at cols
            # (sc*2+hh)*65 .. +64; col +64 holds 1.0 (denominator trick)
            V = [[qkvp.tile([KC, NCS * 2 * (DH + 1)], bf16, tag=f"V{hp}_{s}",
                            name=f"V{hp}_{s}")
                  for s in range(NSB)] for hp in range(NP)]
            for hp in range(NP):
                for s in range(NSB):
                    vv = V[hp][s][:].rearrange("p (k h c) -> p k h c", h=2,
                                               c=DH + 1)
                    nc.vector.memset(vv[:, :, :, DH], 1.0)
            # O[hp]: rows 0:64 = even head out^T (normalized), 64:128 odd
            O = [op.tile([KC, S], bf16, tag=f"O{hp}", name=f"O{hp}")
                 for hp in range(NP)]

            # ---- phase 1: projections ------------------------------------
            with (
                tc.tile_pool(name="pjw", bufs=1) as wp,
                tc.tile_pool(name="xbuf", bufs=1) as xp,
                tc.tile_pool(name="psum_proj", bufs=1, space="PSUM") as pp,
            ):
                # wide tiles, one coarse DMA per contiguous DRAM block
                wqkt = [wp.tile([KC, NDC * 2 * KC], bf16, tag=f"wqk{hp}",
                                name=f"wqk{hp}") for hp in range(NP)]
                wvt = wp.tile([KC, NDC * 2 * KC], bf16, tag="wvt")
                xsb = [xp.tile([KC, NDC * SB], bf16, tag=f"x{s}", name=f"x{s}")
                       for s in range(NSB)]

                HB = NDC // 2 * SB  # columns per x half-block

                def xhalf(s, h):
                    return xsb[s][:, h * HB:(h + 1) * HB]

                nc.sync.dma_start(out=xhalf(0, 0), in_=xt[0, 0])
                nc.gpsimd.dma_start(out=wqkt[0][:], in_=wqk[0])
                nc.sync.dma_start(out=xhalf(0, 1), in_=xt[0, 1])
                nc.gpsimd.dma_start(out=wqkt[1][:], in_=wqk[1])
                nc.sync.dma_start(out=wvt[:], in_=wv[:])
                nc.gpsimd.dma_start(out=xhalf(1, 0), in_=xt[1, 0])
                nc.scalar.dma_start(out=wout_t[0][:], in_=wout[0])
                nc.scalar.dma_start(out=wout_t[1][:], in_=wout[1])
                nc.gpsimd.dma_start(out=xhalf(1, 1), in_=xt[1, 1])
                nc.sync.dma_start(out=xhalf(2, 0), in_=xt[2, 0])
                nc.gpsimd.dma_start(out=xhalf(2, 1), in_=xt[2, 1])
                nc.sync.dma_start(out=xhalf(3, 0), in_=xt[3, 0])
                nc.gpsimd.dma_start(out=xhalf(3, 1), in_=xt[3, 1])

                # warm the ScalarE Exp table so the first attention exp
                # starts instantly
                warm = mp.tile([1, 4], f32, tag="warm")
                nc.vector.memset(warm[:], 0.0)
                nc.scalar.activation(warm[:], warm[:],
                                     mybir.ActivationFunctionType.Exp)

                def emit_qk(sblk):
                    for hp in range(NP):
                        # m-chunk 0 -> QQ[hp], 1 -> KK[hp]
                        for m in range(2):
                            ps = pp.tile([KC, SB], f32, tag="ps_qk", bufs=4)
                            for k in range(NDC):
                                nc.tensor.matmul(
                                    ps[:],
                                    wqkt[hp][:, k * 2 * KC + m * KC:
                                             k * 2 * KC + (m + 1) * KC],
                                    xsb[sblk][:, k * SB:(k + 1) * SB],
                                    start=(k == 0),
                                    stop=(k == NDC - 1),
                                )
                            dest = (QQ if m == 0 else KK)[hp][sblk]
                            nc.vector.tensor_scalar_add(
                                dest[:], ps[:], bqk_t[hp][m][:],
                            )

                def emit_v(sblk):
                    # V natural: lhsT = x^T chunk; rhs = Wv [128, 256]
                    for sc in range(NCS):
                        ps = pp.tile([KC, 2 * KC], f32, tag="ps_v", bufs=4)
                        for k in range(NDC):
                            nc.tensor.matmul(
                                ps[:],
                                xsb[sblk][:, k * SB + sc * KC:
                                          k * SB + (sc + 1) * KC],
                                wvt[:, k * 2 * KC:(k + 1) * 2 * KC],
                                start=(k == 0),
                                stop=(k == NDC - 1),
                            )
                        for hp in range(NP):
                            vslc = V[hp][sblk][:, sc * 2 * (DH + 1):
                                               (sc + 1) * 2 * (DH + 1)]
                            vv = vslc.rearrange("p (h c) -> p h c", c=DH + 1)
                            ps2 = ps[:, hp * KC:(hp + 1) * KC].rearrange(
                                "p (h c) -> p h c", c=DH)
                            bv2 = bv_bc[:, hp * KC:(hp + 1) * KC].rearrange(
                                "p (h c) -> p h c", c=DH)
                            nc.vector.tensor_add(vv[:, :, 0:DH], ps2[:], bv2[:])

                # QK of sblk0+1 first: ~18us of PE work that only needs
                # x/wqk, giving the wv / sblk1 x DMAs arrival slack
                emit_qk(0)
                emit_qk(1)
                emit_v(0)
                emit_v(1)
                emit_qk(2)
                emit_v(2)
                emit_qk(3)
                emit_v(3)

            # ---- phase 2: attention + interleaved partial out-proj -------
            with (
                tc.tile_pool(name="pbuf", bufs=1) as pb,
                tc.tile_pool(name="psum_att", bufs=1, space="PSUM") as pa,
            ):
                def make_scores(P, hp, qblk, kc):
                    def emit():
                        d = kc - 4 * qblk
                        # causal: cols < 128*d are fully masked; skip them
                        c0 = KC * max(d, 0)
                        ps = pa.tile([KC, 2 * SB], f32, tag="ps_s", bufs=2)
                        for hh in range(2):  # row-tiled head pair
                            r0 = hh * DH
                            nc.tensor.matmul(
                                ps[:, hh * SB + c0:(hh + 1) * SB],
                                KK[hp][kc // 4][r0:r0 + DH,
                                                (kc % 4) * KC:
                                                (kc % 4 + 1) * KC],
                                QQ[hp][qblk][r0:r0 + DH, c0:SB],
                                start=True,
                                stop=True,
                            )
                        pd3 = P[kc][:].rearrange("p (h f) -> p h f", h=2)
                        if c0 == 0:
                            nc.scalar.activation(
                                P[kc][:],
                                ps[:],
                                mybir.ActivationFunctionType.Exp,
                                scale=1.0 / float(np.sqrt(DH)),
                            )
                        else:
                            ps3 = ps[:].rearrange("p (h f) -> p h f", h=2)
                            nc.scalar.activation(
                                pd3[:, :, c0:SB],
                                ps3[:, :, c0:SB],
                                mybir.ActivationFunctionType.Exp,
                                scale=1.0 / float(np.sqrt(DH)),
                            )
                        if d >= 0:  # diagonal chunk: zero where k > q
                            nc.gpsimd.affine_select(
                                out=pd3[:, :, c0:c0 + KC],
                                in_=pd3[:, :, c0:c0 + KC],
                                pattern=[[0, 2], [1, KC]],
                                compare_op=mybir.AluOpType.is_ge,
                                fill=0.0,
                                base=0,
                                channel_multiplier=-1,
                            )
                    return emit

                def make_av(P, pos, hp, qblk, kc, nkc):
                    def emit():
                        d = kc - 4 * qblk
                        c0 = KC * max(d, 0)
                        for hh in range(2):
                            nc.tensor.matmul(
                                pos[hh][:, c0:SB],
                                V[hp][kc // 4][:,
                                    ((kc % 4) * 2 + hh) * (DH + 1):
                                    ((kc % 4) * 2 + hh + 1) * (DH + 1)],
                                P[kc][:, hh * SB + c0:(hh + 1) * SB],
                                start=(kc == 0),
                                stop=(kc == nkc - 1),
                            )
                    return emit

                def make_norm(pos, hp, qblk):
                    def emit():
                        # den row first (own base-partition-0 tile: custom
                        # DVE ops key on partition 0), then its reciprocal,
                        # then the bulk eviction -- pos banks free after the
                        # copies; recip/broadcast/mult run off the PE path
                        dens, rdens, posb = [], [], []
                        for hh in range(2):
                            dn = ep.tile([1, SB], f32, tag=f"den{hh}",
                                         bufs=2, name=f"den{hh}_{hp}_{qblk}")
                            nc.vector.tensor_copy(dn[:], pos[hh][DH:DH + 1, :])
                            dens.append(dn)
                            rden = ep.tile([1, SB], f32, tag=f"rden{hh}",
                                           bufs=2, name=f"rden{hh}_{hp}_{qblk}")
                            nc.vector.reciprocal_approx_fast(
                                out=rden[:], in_=dn[:])
                            rdens.append(rden)
                        for hh in range(2):
                            pb_t = ep.tile([DH, SB], f32, tag=f"posb{hh}",
                                           bufs=2, name=f"posb{hh}_{hp}_{qblk}")
                            nc.vector.tensor_copy(pb_t[:], pos[hh][0:DH, :])
                            posb.append(pb_t)
                        for hh in range(2):
                            rden_bc = ep.tile([DH, SB], f32, tag="rden_bc",
                                              bufs=2)
                            nc.gpsimd.partition_broadcast(
                                out_ap=rden_bc[:], in_ap=rdens[hh][:])
                            r0 = hh * DH
                            nc.vector.tensor_mul(
                                O[hp][r0:r0 + DH, qblk * SB:(qblk + 1) * SB],
                                posb[hh][:],
                                rden_bc[:],
                            )
                    return emit

                def make_outproj_units(qblk):
                    units = []
                    otiles = {}

                    def unit(tc_, nb):
                        def emit():
                            t0 = qblk * SB + tc_ * KC
                            if nb == 0:
                                otiles[tc_] = ep.tile(
                                    [KC, D], bf16, tag="osb", bufs=2,
                                    name=f"osb{qblk}_{tc_}")
                            pso = pa.tile([KC, SB], f32, tag="ps_o", bufs=2,
                                          name=f"pso{qblk}_{tc_}_{nb}")
                            for hp in range(NP):
                                nc.tensor.matmul(
                                    pso[:],
                                    O[hp][:, t0:t0 + KC],
                                    wout_t[hp][:, nb * SB:(nb + 1) * SB],
                                    start=(hp == 0),
                                    stop=(hp == NP - 1),
                                )
                            nc.vector.tensor_copy(
                                otiles[tc_][:, nb * SB:(nb + 1) * SB], pso[:])
                            if nb == 1:
                                nc.sync.dma_start(
                                    out=out_ext[qblk * NCS + tc_],
                                    in_=otiles[tc_][:],
                                )
                        return emit

                    for tc_ in range(NCS):
                        for nb in range(2):
                            units.append(unit(tc_, nb))
                    return units

                # one global slot stream: scores lead, AV trails by LAG
                # chunks and drains across head-pair and q-block boundaries
                score_emits = []
                av_emits = []
                qblk_of_slot = []
                for qblk in range(NSB):
                    nkc = 4 * (qblk + 1)
                    for hp in range(NP):
                        P = [pb.tile([KC, 2 * SB], bf16, tag=f"P{kc}",
                                     name=f"P{kc}_{hp}_{qblk}", bufs=2)
                             for kc in range(nkc)]
                        pos = [pa.tile([DH + 1, SB], f32, tag=f"ps_av{hh}",
                                       bufs=1, name=f"po{hh}_{hp}_{qblk}")
                               for hh in range(2)]
                        norm = make_norm(pos, hp, qblk)
                        for kc in range(nkc):
                            score_emits.append(make_scores(P, hp, qblk, kc))
                            av = make_av(P, pos, hp, qblk, kc, nkc)
                            if kc == nkc - 1:
                                av_emits.append(
                                    (lambda a, n: lambda: (a(), n()))(av, norm))
                            else:
                                av_emits.append(av)
                            qblk_of_slot.append(qblk)

                n_slots = len(score_emits)
                pending = []
                cur_qblk = 0
                slot_in_qblk = 0
                for slot in range(n_slots + LAG):
                    if slot < n_slots:
                        if qblk_of_slot[slot] != cur_qblk:
                            while pending:
                                pending.pop(0)()
                            cur_qblk = qblk_of_slot[slot]
                            slot_in_qblk = 0
                            pending = make_outproj_units(cur_qblk - 1)
                        score_emits[slot]()
                        if pending and slot_in_qblk >= 4:
                            pending.pop(0)()
                        slot_in_qblk += 1
                    if slot >= LAG:
                        av_emits[slot - LAG]()
                while pending:
                    pending.pop(0)()
                for u in make_outproj_units(NSB - 1):
                    u()

    nc.compile()
    return nc


def _get_program():
    global _compiled
    if _compiled is None:
        _compiled = _build()
    return _compiled


def _shard_inputs(x, Wqkv, bqkv, Wout, bout):
    """Build the 8 per-core input maps (all host-side numpy)."""
    bf = ml_dtypes.bfloat16
    x = np.asarray(x, dtype=np.float32)
    Wqkv = np.asarray(Wqkv, dtype=np.float32)
    bqkv = np.ascontiguousarray(np.asarray(bqkv, dtype=np.float32))
    Wout = np.asarray(Wout, dtype=np.float32)

    Wq = Wqkv[:, 0 * D:1 * D]
    Wk = Wqkv[:, 1 * D:2 * D]
    Wv_full = Wqkv[:, 2 * D:3 * D]
    bq = bqkv[0 * D:1 * D]
    bk = bqkv[1 * D:2 * D]
    bv_full = bqkv[2 * D:3 * D]

    # p-major halves: xt[s, h][p, kk*SB+f] = x^T[(4h+kk)*128+p, s*SB+f]
    xt_b = [np.ascontiguousarray(
        x[b].T.reshape(2, 4, KC, NSB, SB).transpose(3, 0, 2, 1, 4)
            .reshape(NSB, 2, KC, NDC // 2 * SB).astype(bf)) for b in range(B)]

    def pmaj_w(w):  # [D, C] -> [KC, NDC*C], chunk-major columns
        c = w.shape[1]
        return np.ascontiguousarray(
            w.reshape(NDC, KC, c).transpose(1, 0, 2).reshape(KC, NDC * c))

    in_maps = []
    for c in range(NCORES):
        g, i = c // 4, c % 4
        hs = 4 * i * DH           # first head-dim column of my 4 heads
        wqk_c = np.stack([
            pmaj_w(np.concatenate(
                [Wq[:, hs + (2 * hp) * DH: hs + (2 * hp + 1) * DH],
                 Wq[:, hs + (2 * hp + 1) * DH: hs + (2 * hp + 2) * DH],
                 Wk[:, hs + (2 * hp) * DH: hs + (2 * hp + 1) * DH],
                 Wk[:, hs + (2 * hp + 1) * DH: hs + (2 * hp + 2) * DH]],
                axis=1))
            for hp in range(NP)])
        bqk_c = np.stack([
            np.stack([
                np.concatenate([bq[hs + (2 * hp) * DH: hs + (2 * hp + 1) * DH],
                                bq[hs + (2 * hp + 1) * DH:
                                   hs + (2 * hp + 2) * DH]]),
                np.concatenate([bk[hs + (2 * hp) * DH: hs + (2 * hp + 1) * DH],
                                bk[hs + (2 * hp + 1) * DH:
                                   hs + (2 * hp + 2) * DH]])])
            for hp in range(NP)])
        wv_c = pmaj_w(Wv_full[:, hs:hs + 4 * DH]).astype(bf)
        bv_c = np.ascontiguousarray(bv_full[hs:hs + 4 * DH])
        wout_c = np.ascontiguousarray(
            Wout[hs:hs + 4 * DH].reshape(NP, KC, D).astype(bf))
        in_maps.append({
            "xt": xt_b[g],
            "wqk": np.ascontiguousarray(wqk_c.astype(bf)),
            "wv": wv_c,
            "wout": wout_c,
            "bqk": np.ascontiguousarray(bqk_c.astype(np.float32)),
            "bv": bv_c,
        })
    return in_maps


def run(inputs, trace=False, trace_kwargs=None):
    nc = _get_program()
    in_maps = _shard_inputs(**inputs)
    res = run_bass_kernel_spmd(
        nc, in_maps, list(range(NCORES)), trace=trace,
        **(trace_kwargs or {}),
    )
    bout = np.asarray(inputs["bout"], dtype=np.float32)
    out = np.empty((B, S, D), dtype=np.float32)
    for b in range(B):
        acc = np.zeros((S, D), dtype=np.float32)
        for i in range(4):
            acc += np.asarray(res.results[4 * b + i]["out"],
                              dtype=np.float32).reshape(S, D)
        out[b] = acc + bout
    return out, res


def kernel(**inputs):
    out, _ = run(inputs)
    return out


# revision 35
# speedup vs baseline: 1.2703x; 1.0030x over previous
"""Causal multi-head attention (B=2, S=2048, D=1024, H=16) on 8 trn2 cores.

Sharding: core c handles heads {4i..4i+3} (i = c%4) of batch c//4 only.
All matmul operands are bfloat16 (PSUM accumulates fp32). The output
projection is row-parallel: each core multiplies its own heads' attention
output by the matching 256 rows of Wout, producing a bf16 partial
[2048, 1024] for its batch; the host unshards by summing the four
partials per batch and adding bout. No device collective is needed.

Per core:
  - project host-pretransposed x_b^T [D, S] through the core's Wqkv
    column slice into Q^T/K^T head-pair tiles and V (natural layout with
    a fused ones-column so the AV matmul emits softmax denominators),
  - causal attention per head pair in transposed layout: scores^T = K Q^T
    (row-tiled head pair), exp on ScalarE, causal diagonal-band masks via
    gpsimd affine_select, A^T V on PE, normalization via fast reciprocal.

Schedule notes (all measured on hw traces):
  - the TRN2 PE has a p-state ramp (0.65/1.2/2.4 GHz, max only after
    ~3us continuously busy), so PE gaps cost ~3x their length; the whole
    phase-2 stream is emitted as one global scores/AV slot queue where
    AV trails scores by LAG chunks and drains across head-pair and
    q-block boundaries,
  - the ScalarE exp stream (~74us) is within a few % of the attention
    PE work; one exp per chunk ([128, 2, f]) -- per-hh splits add
    +140ns/instr and make exp the pacer,
  - AV accumulators are evicted to SBUF immediately (copies) so the
    single-buffered pos PSUM banks free fast; the reciprocal/broadcast/
    multiply run lazily off the PE critical path. Custom DVE ops key on
    partition 0, so the denominator row gets its own [1, SB] tile,
  - phase-1 inputs arrive via ~12 coarse DMAs (contiguous 0.5-1MB
    blocks into wide SBUF tiles): 56 small DMAs were issue-rate bound
    (~640ns descriptor-gen each) and starved the first QK chains.
"""

import sys

for _p in ("/opt/trn_rl_repo", "/opt/pypackages"):
    if _p not in sys.path:
        sys.path.insert(0, _p)

import numpy as np
import ml_dtypes

import concourse.bass as bass
import concourse.mybir as mybir
import concourse.tile as tile
from concourse import bacc
from concourse.bass_utils import run_bass_kernel_spmd

B = 2
S = 2048
D = 1024
H = 16
DH = 64
NCORES = 8
SB = 512           # q block (matmul moving dim)
KC = 128           # k chunk (contraction tile)
NSB = S // SB      # 4 q-blocks
NKC = S // KC      # 16 k-chunks
NDC = D // KC      # 8 contraction chunks for the projections
NCS = SB // KC     # 4 k-chunks per sequence block
NP = 2             # head pairs per core
LAG = 4            # AV trails scores by LAG chunks in the PE stream

_compiled = None


def _build():
    f32 = mybir.dt.float32
    bf16 = mybir.dt.bfloat16
    nc = bacc.Bacc(None, target_bir_lowering=False)

    # host-preblocked inputs, p-major so each DMA is contiguous on BOTH sides:
    # xt[s, j] is a [128, 1024] quarter (d-chunks 2j, 2j+1 side by side)
    xt = nc.declare_dram_parameter("xt", [NSB, 4, KC, 2 * SB], bf16,
                                   isOutput=False)
    wqk = nc.declare_dram_parameter("wqk", [NP, 2, KC, NDC * KC], bf16,
                                    isOutput=False)
    wv = nc.declare_dram_parameter("wv", [KC, NDC * 2 * KC], bf16, isOutput=False)
    wout = nc.declare_dram_parameter("wout", [NP, KC, D], bf16, isOutput=False)
    bqk = nc.declare_dram_parameter("bqk", [NP, 2, KC], f32, isOutput=False)
    bv = nc.declare_dram_parameter("bv", [2 * KC], f32, isOutput=False)
    # blocked [16, 128, 1024]: one contiguous 256KB block per token chunk
    out_ext = nc.declare_dram_parameter("out", [NSB * NCS, KC, D], bf16,
                                        isOutput=True)

    with tile.TileContext(nc) as tc:
        with (
            tc.tile_pool(name="qkv", bufs=1) as qkvp,
            tc.tile_pool(name="obuf", bufs=1) as op,
            tc.tile_pool(name="misc", bufs=1) as mp,
            tc.tile_pool(name="evict", bufs=1) as ep,
        ):
            # ---- small constants -----------------------------------------
            bqk_t = [[mp.tile([KC, 1], f32, tag=f"bqk{hp}_{m}",
                              name=f"bqk{hp}_{m}") for m in range(2)]
                     for hp in range(NP)]
            for hp in range(NP):
                for m in range(2):
                    nc.scalar.dma_start(
                        out=bqk_t[hp][m][:],
                        in_=bqk[hp, m].rearrange("(p o) -> p o", o=1),
                    )
            bv_row = mp.tile([1, 2 * KC], f32, tag="bv_row")
            nc.scalar.dma_start(out=bv_row[:], in_=bv.rearrange("(o f) -> o f", o=1))
            bv_bc = mp.tile([KC, 2 * KC], f32, tag="bv_bc")
            nc.gpsimd.partition_broadcast(out_ap=bv_bc[:], in_ap=bv_row[:])

            # Wout row slices for my two head pairs
            wout_t = [mp.tile([KC, D], bf16, tag=f"wo{hp}", name=f"wo{hp}")
                      for hp in range(NP)]

            # ---- persistent activations ----------------------------------
            # pair hp = heads (4i+2hp, 4i+2hp+1) of my batch.
            # QQ[hp]: rows 0:64 = Q^T of the even head, rows 64:128 odd head
            QQ = [[qkvp.tile([KC, SB], bf16, tag=f"QQ{hp}_{s}", name=f"QQ{hp}_{s}")
                   for s in range(NSB)] for hp in range(NP)]
            KK = [[qkvp.tile([KC, SB], bf16, tag=f"KK{hp}_{s}", name=f"KK{hp}_{s}")
                   for s in range(NSB)] for hp in range(NP)]
            # V[hp][s]: [128, 4*2*65]; chunk sc head hh at cols
            # (sc*2+hh)*65 .. +64; col +64 holds 1.0 (denominator trick)
            V = [[qkvp.tile([KC, NCS * 2 * (DH + 1)], bf16, tag=f"V{hp}_{s}",
                            name=f"V{hp}_{s}")
                  for s in range(NSB)] for hp in range(NP)]
            for hp in range(NP):
                for s in range(NSB):
                    vv = V[hp][s][:].rearrange("p (k h c) -> p k h c", h=2,
                                               c=DH + 1)
                    nc.vector.memset(vv[:, :, :, DH], 1.0)
            # O[hp]: rows 0:64 = even head out^T (normalized), 64:128 odd
            O = [op.tile([KC, S], bf16, tag=f"O{hp}", name=f"O{hp}")
                 for hp in range(NP)]

            # ---- phase 1: projections ------------------------------------
            with (
                tc.tile_pool(name="pjw", bufs=1) as wp,
                tc.tile_pool(name="xbuf", bufs=1) as xp,
                tc.tile_pool(name="psum_proj", bufs=1, space="PSUM") as pp,
            ):
                # x as 4 quarter-tiles per sblk, wqk as 2 half-tiles per hp:
                # small separate first tiles so the first QK chain's deps
                # arrive right after the fixed ~9us framework startup
                wqkt = [[wp.tile([KC, NDC * KC], bf16, tag=f"wqk{hp}_{h}",
                                 name=f"wqk{hp}_{h}") for h in range(2)]
                        for hp in range(NP)]
                wvt = wp.tile([KC, NDC * 2 * KC], bf16, tag="wvt")
                xq = [[xp.tile([KC, 2 * SB], bf16, tag=f"x{s}_{j}",
                               name=f"x{s}_{j}") for j in range(4)]
                      for s in range(NSB)]

                def xchunk(s, k):  # [128, 512] rhs slice for d-chunk k
                    return xq[s][k // 2][:, (k % 2) * SB:(k % 2 + 1) * SB]

                def wqkchunk(hp, k, m):  # [128, 128] lhsT slice, chunk k
                    c0 = (k % 4) * 2 * KC + m * KC
                    return wqkt[hp][k // 4][:, c0:c0 + KC]

                for j in range(4):
                    nc.sync.dma_start(out=xq[0][j][:], in_=xt[0, j])
                    hp_, h_ = divmod(j, 2)
                    eng = nc.gpsimd if hp_ == 0 else nc.scalar
                    eng.dma_start(out=wqkt[hp_][h_][:], in_=wqk[hp_, h_])
                nc.gpsimd.dma_start(out=wvt[:], in_=wv[:])
                for j in range(4):
                    (nc.gpsimd if j % 2 else nc.sync).dma_start(
                        out=xq[1][j][:], in_=xt[1, j])
                nc.scalar.dma_start(out=wout_t[0][:], in_=wout[0])
                nc.scalar.dma_start(out=wout_t[1][:], in_=wout[1])
                for s in range(2, NSB):
                    for j in range(4):
                        (nc.gpsimd if j % 2 else nc.sync).dma_start(
                            out=xq[s][j][:], in_=xt[s, j])

                # warm the ScalarE Exp table so the first attention exp
                # starts instantly
                warm = mp.tile([1, 4], f32, tag="warm")
                nc.vector.memset(warm[:], 0.0)
                nc.scalar.activation(warm[:], warm[:],
                                     mybir.ActivationFunctionType.Exp)

                def emit_qk(sblk):
                    for hp in range(NP):
                        # m-chunk 0 -> QQ[hp], 1 -> KK[hp]
                        for m in range(2):
                            ps = pp.tile([KC, SB], f32, tag="ps_qk", bufs=4)
                            for k in range(NDC):
                                nc.tensor.matmul(
                                    ps[:],
                                    wqkchunk(hp, k, m),
                                    xchunk(sblk, k),
                                    start=(k == 0),
                                    stop=(k == NDC - 1),
                                )
                            dest = (QQ if m == 0 else KK)[hp][sblk]
                            nc.vector.tensor_scalar_add(
                                dest[:], ps[:], bqk_t[hp][m][:],
                            )

                def emit_v(sblk):
                    # V natural: lhsT = x^T chunk; rhs = Wv [128, 256]
                    for sc in range(NCS):
                        ps = pp.tile([KC, 2 * KC], f32, tag="ps_v", bufs=4)
                        for k in range(NDC):
                            c0 = (k % 2) * SB + sc * KC
                            nc.tensor.matmul(
                                ps[:],
                                xq[sblk][k // 2][:, c0:c0 + KC],
                                wvt[:, k * 2 * KC:(k + 1) * 2 * KC],
                                start=(k == 0),
                                stop=(k == NDC - 1),
                            )
                        for hp in range(NP):
                            vslc = V[hp][sblk][:, sc * 2 * (DH + 1):
                                               (sc + 1) * 2 * (DH + 1)]
                            vv = vslc.rearrange("p (h c) -> p h c", c=DH + 1)
                            ps2 = ps[:, hp * KC:(hp + 1) * KC].rearrange(
                                "p (h c) -> p h c", c=DH)
                            bv2 = bv_bc[:, hp * KC:(hp + 1) * KC].rearrange(
                                "p (h c) -> p h c", c=DH)
                            nc.vector.tensor_add(vv[:, :, 0:DH], ps2[:], bv2[:])

                # QK of sblk0+1 first: ~18us of PE work that only needs
                # x/wqk, giving the wv / sblk1 x DMAs arrival slack
                emit_qk(0)
                emit_qk(1)
                emit_v(0)
                emit_v(1)
                emit_qk(2)
                emit_v(2)
                emit_qk(3)
                emit_v(3)

            # ---- phase 2: attention + interleaved partial out-proj -------
            with (
                tc.tile_pool(name="pbuf", bufs=1) as pb,
                tc.tile_pool(name="psum_att", bufs=1, space="PSUM") as pa,
            ):
                def make_scores(P, hp, qblk, kc):
                    def emit():
                        d = kc - 4 * qblk
                        # causal: cols < 128*d are fully masked; skip them
                        c0 = KC * max(d, 0)
                        ps = pa.tile([KC, 2 * SB], f32, tag="ps_s", bufs=2)
                        for hh in range(2):  # row-tiled head pair
                            r0 = hh * DH
                            nc.tensor.matmul(
                                ps[:, hh * SB + c0:(hh + 1) * SB],
                                KK[hp][kc // 4][r0:r0 + DH,
                                                (kc % 4) * KC:
                                                (kc % 4 + 1) * KC],
                                QQ[hp][qblk][r0:r0 + DH, c0:SB],
                                start=True,
                                stop=True,
                            )
                        pd3 = P[kc][:].rearrange("p (h f) -> p h f", h=2)
                        if c0 == 0:
                            nc.scalar.activation(
                                P[kc][:],
                                ps[:],
                                mybir.ActivationFunctionType.Exp,
                                scale=1.0 / float(np.sqrt(DH)),
                            )
                        else:
                            ps3 = ps[:].rearrange("p (h f) -> p h f", h=2)
                            nc.scalar.activation(
                                pd3[:, :, c0:SB],
                                ps3[:, :, c0:SB],
                                mybir.ActivationFunctionType.Exp,
                                scale=1.0 / float(np.sqrt(DH)),
                            )
                        if d >= 0:  # diagonal chunk: zero where k > q
                            nc.gpsimd.affine_select(
                                out=pd3[:, :, c0:c0 + KC],
                                in_=pd3[:, :, c0:c0 + KC],
                                pattern=[[0, 2], [1, KC]],
                                compare_op=mybir.AluOpType.is_ge,
                                fill=0.0,
                                base=0,
                                channel_multiplier=-1,
                            )
                    return emit

                def make_av(P, pos, hp, qblk, kc, nkc):
                    def emit():
                        d = kc - 4 * qblk
                        c0 = KC * max(d, 0)
                        for hh in range(2):
                            nc.tensor.matmul(
                                pos[hh][:, c0:SB],
                                V[hp][kc // 4][:,
                                    ((kc % 4) * 2 + hh) * (DH + 1):
                                    ((kc % 4) * 2 + hh + 1) * (DH + 1)],
                                P[kc][:, hh * SB + c0:(hh + 1) * SB],
                                start=(kc == 0),
                                stop=(kc == nkc - 1),
                            )
                    return emit

                def make_norm(pos, hp, qblk):
                    def emit():
                        # den row first (own base-partition-0 tile: custom
                        # DVE ops key on partition 0), then its reciprocal,
                        # then the bulk eviction -- pos banks free after the
                        # copies; recip/broadcast/mult run off the PE path
                        dens, rdens, posb = [], [], []
                        for hh in range(2):
                            dn = ep.tile([1, SB], f32, tag=f"den{hh}",
                                         bufs=2, name=f"den{hh}_{hp}_{qblk}")
                            nc.vector.tensor_copy(dn[:], pos[hh][DH:DH + 1, :])
                            dens.append(dn)
                            rden = ep.tile([1, SB], f32, tag=f"rden{hh}",
                                           bufs=2, name=f"rden{hh}_{hp}_{qblk}")
                            nc.vector.reciprocal_approx_fast(
                                out=rden[:], in_=dn[:])
                            rdens.append(rden)
                        for hh in range(2):
                            pb_t = ep.tile([DH, SB], f32, tag=f"posb{hh}",
                                           bufs=2, name=f"posb{hh}_{hp}_{qblk}")
                            nc.vector.tensor_copy(pb_t[:], pos[hh][0:DH, :])
                            posb.append(pb_t)
                        for hh in range(2):
                            rden_bc = ep.tile([DH, SB], f32, tag="rden_bc",
                                              bufs=2)
                            nc.gpsimd.partition_broadcast(
                                out_ap=rden_bc[:], in_ap=rdens[hh][:])
                            r0 = hh * DH
                            nc.vector.tensor_mul(
                                O[hp][r0:r0 + DH, qblk * SB:(qblk + 1) * SB],
                                posb[hh][:],
                                rden_bc[:],
                            )
                    return emit

                def make_outproj_units(qblk):
                    units = []
                    otiles = {}

                    def unit(tc_, nb):
                        def emit():
                            t0 = qblk * SB + tc_ * KC
                            if nb == 0:
                                otiles[tc_] = ep.tile(
                                    [KC, D], bf16, tag="osb", bufs=2,
                                    name=f"osb{qblk}_{tc_}")
                            pso = pa.tile([KC, SB], f32, tag="ps_o", bufs=2,
                                          name=f"pso{qblk}_{tc_}_{nb}")
                            for hp in range(NP):
                                nc.tensor.matmul(
                                    pso[:],
                                    O[hp][:, t0:t0 + KC],
                                    wout_t[hp][:, nb * SB:(nb + 1) * SB],
                                    start=(hp == 0),
                                    stop=(hp == NP - 1),
                                )
                            nc.vector.tensor_copy(
                                otiles[tc_][:, nb * SB:(nb + 1) * SB], pso[:])
                            if nb == 1:
                                nc.sync.dma_start(
                                    out=out_ext[qblk * NCS + tc_],
                                    in_=otiles[tc_][:],
                                )
                        return emit

                    for tc_ in range(NCS):
                        for nb in range(2):
                            units.append(unit(tc_, nb))
                    return units

                # one global slot stream: scores lead, AV trails by LAG
                # chunks and drains across head-pair and q-block boundaries
                score_emits = []
                av_emits = []
                qblk_of_slot = []
                for qblk in range(NSB):
                    nkc = 4 * (qblk + 1)
                    for hp in range(NP):
                        P = [pb.tile([KC, 2 * SB], bf16, tag=f"P{kc}",
                                     name=f"P{kc}_{hp}_{qblk}", bufs=2)
                             for kc in range(nkc)]
                        pos = [pa.tile([DH + 1, SB], f32, tag=f"ps_av{hh}",
                                       bufs=1, name=f"po{hh}_{hp}_{qblk}")
                               for hh in range(2)]
                        norm = make_norm(pos, hp, qblk)
                        for kc in range(nkc):
                            score_emits.append(make_scores(P, hp, qblk, kc))
                            av = make_av(P, pos, hp, qblk, kc, nkc)
                            if kc == nkc - 1:
                                av_emits.append(
                                    (lambda a, n: lambda: (a(), n()))(av, norm))
                            else:
                                av_emits.append(av)
                            qblk_of_slot.append(qblk)

                n_slots = len(score_emits)
                pending = []
                cur_qblk = 0
                slot_in_qblk = 0
                for slot in range(n_slots + LAG):
                    if slot < n_slots:
                        if qblk_of_slot[slot] != cur_qblk:
                            while pending:
                                pending.pop(0)()
                            cur_qblk = qblk_of_slot[slot]
                            slot_in_qblk = 0
                            pending = make_outproj_units(cur_qblk - 1)
                        score_emits[slot]()
                        if pending and slot_in_qblk >= 4:
                            pending.pop(0)()
                        slot_in_qblk += 1
                    if slot >= LAG:
                        av_emits[slot - LAG]()
                while pending:
                    pending.pop(0)()
                for u in make_outproj_units(NSB - 1):
                    u()

    nc.compile()
    return nc


def _get_program():
    global _compiled
    if _compiled is None:
        _compiled = _build()
    return _compiled


def _shard_inputs(x, Wqkv, bqkv, Wout, bout):
    """Build the 8 per-core input maps (all host-side numpy)."""
    bf = ml_dtypes.bfloat16
    x = np.asarray(x, dtype=np.float32)
    Wqkv = np.asarray(Wqkv, dtype=np.float32)
    bqkv = np.ascontiguousarray(np.asarray(bqkv, dtype=np.float32))
    Wout = np.asarray(Wout, dtype=np.float32)

    Wq = Wqkv[:, 0 * D:1 * D]
    Wk = Wqkv[:, 1 * D:2 * D]
    Wv_full = Wqkv[:, 2 * D:3 * D]
    bq = bqkv[0 * D:1 * D]
    bk = bqkv[1 * D:2 * D]
    bv_full = bqkv[2 * D:3 * D]

    # p-major quarters: xt[s, j][p, kk*SB+f] = x^T[(2j+kk)*128+p, s*SB+f]
    xt_b = [np.ascontiguousarray(
        x[b].T.reshape(4, 2, KC, NSB, SB).transpose(3, 0, 2, 1, 4)
            .reshape(NSB, 4, KC, 2 * SB).astype(bf)) for b in range(B)]

    def pmaj_w(w):  # [D, C] -> [KC, NDC*C], chunk-major columns
        c = w.shape[1]
        return np.ascontiguousarray(
            w.reshape(NDC, KC, c).transpose(1, 0, 2).reshape(KC, NDC * c))

    in_maps = []
    for c in range(NCORES):
        g, i = c // 4, c % 4
        hs = 4 * i * DH           # first head-dim column of my 4 heads
        wqk_c = np.stack([
            pmaj_w(np.concatenate(
                [Wq[:, hs + (2 * hp) * DH: hs + (2 * hp + 1) * DH],
                 Wq[:, hs + (2 * hp + 1) * DH: hs + (2 * hp + 2) * DH],
                 Wk[:, hs + (2 * hp) * DH: hs + (2 * hp + 1) * DH],
                 Wk[:, hs + (2 * hp + 1) * DH: hs + (2 * hp + 2) * DH]],
                axis=1)).reshape(KC, 2, NDC * KC).transpose(1, 0, 2)
            for hp in range(NP)])
        bqk_c = np.stack([
            np.stack([
                np.concatenate([bq[hs + (2 * hp) * DH: hs + (2 * hp + 1) * DH],
                                bq[hs + (2 * hp + 1) * DH:
                                   hs + (2 * hp + 2) * DH]]),
                np.concatenate([bk[hs + (2 * hp) * DH: hs + (2 * hp + 1) * DH],
                                bk[hs + (2 * hp + 1) * DH:
                                   hs + (2 * hp + 2) * DH]])])
            for hp in range(NP)])
        wv_c = pmaj_w(Wv_full[:, hs:hs + 4 * DH]).astype(bf)
        bv_c = np.ascontiguousarray(bv_full[hs:hs + 4 * DH])
        wout_c = np.ascontiguousarray(
            Wout[hs:hs + 4 * DH].reshape(NP, KC, D).astype(bf))
        in_maps.append({
            "xt": xt_b[g],
            "wqk": np.ascontiguousarray(wqk_c.astype(bf)),
            "wv": wv_c,
            "wout": wout_c,
            "bqk": np.ascontiguousarray(bqk_c.astype(np.float32)),
            "bv": bv_c,
        })
    return in_maps


def run(inputs, trace=False, trace_kwargs=None):
    nc = _get_program()
    in_maps = _shard_inputs(**inputs)
    res = run_bass_kernel_spmd(
        nc, in_maps, list(range(NCORES)), trace=trace,
        **(trace_kwargs or {}),
    )
    bout = np.asarray(inputs["bout"], dtype=np.float32)
    out = np.empty((B, S, D), dtype=np.float32)
    for b in range(B):
        acc = np.zeros((S, D), dtype=np.float32)
        for i in range(4):
            acc += np.asarray(res.results[4 * b + i]["out"],
                              dtype=np.float32).reshape(S, D)
        out[b] = acc + bout
    return out, res


def kernel(**inputs):
    out, _ = run(inputs)
    return out
